# revision 23
# baseline (speedup 1.0000x reference)
"""Trainium2 Bass kernel for nn_MultiHeadAttention_4690104287263.

Strategy (8 NeuronCores, SPMD):
  - Tensor-parallel over heads: each core owns 2 of the 16 heads.
  - Projections computed TRANSPOSED (qT/kT: [dqk, tok]) so the attention
    matmuls need no on-device transposes. RoPE pairs are interleaved into
    adjacent partitions (baked into the host-side weight column order) so
    the half-rotation becomes a DVE stream_shuffle (even/odd lane swap).
  - Attention computed transposed: sT[k, q] = kT.T @ qT, softmax over the
    partition (k) dim using exp (no max subtraction; scores are O(1)) and
    a ones-column appended to V so the PV matmul also yields the softmax
    denominator for free. Block-causal: upper k-tiles skipped, diagonal
    tiles masked multiplicatively after exp.
  - AllToAll (2 calls, one per local head) reshards attention output from
    head-sharded to token-sharded; Wo is then computed token-sharded with
    the full (zero-padded, even/odd-head-grouped) Wo. No all-reduce.
  - bf16 storage/matmuls with f32 PSUM accumulation throughout.
"""

import os
import numpy as np
import ml_dtypes

bf16 = ml_dtypes.bfloat16

B, S, D, H, DQK, DV = 2, 2048, 2048, 16, 128, 85
T = B * S                 # 4096 flat tokens
NCORE = 8
HL = 2                    # heads per core
SCALE = float(DQK) ** -0.5
ROPE_BASE = 10000.0
CW = 512                  # token chunk width
NCHUNK = T // CW          # 8
KT = D // 128             # 16 k-tiles over d_model
NTT = T // 128            # 32 token tiles
DVP = 97                  # dv(85) + pad + ones column at DEN (32-aligned)
DEN = 96                  # denominator row (must be a legal partition start)
NKT_WO = 12               # Wo K-tiles (2 groups x 6, zero padded 680->768)

SWAP_MASK = [i ^ 1 for i in range(32)]

LAST = {"exec_time_ns": None, "results": None}


def _host_inputs(q, k, v, mask, Wq, bq, Wk, bk, Wv, bv, Wo, bo):
    """Prepare per-core input maps (host-side shard/layout/cast)."""
    f32 = np.float32
    # x^T layouts [D, T], bf16
    xq_t = np.ascontiguousarray(q.reshape(T, D).T).astype(bf16)
    xk_t = np.ascontiguousarray(k.reshape(T, D).T).astype(bf16)
    xv_t = np.ascontiguousarray(v.reshape(T, D).T).astype(bf16)

    # rope pair-interleave permutation within each head's 128 cols
    perm = np.empty(128, np.int64)
    perm[0::2] = np.arange(64)
    perm[1::2] = np.arange(64) + 64

    # cos/sin tables in interleaved layout [128, T]
    inv_freq = 1.0 / (ROPE_BASE ** (np.arange(64, dtype=np.float64) / 64.0))
    pos = np.arange(S, dtype=np.float64)
    ang = pos[:, None] * inv_freq[None, :]           # [S, 64]
    cos = np.cos(ang).T                              # [64, S]
    sin = np.sin(ang).T
    cs_cc = np.empty((128, T), f32)
    cs_ss = np.empty((128, T), f32)
    for b in range(B):
        sl = slice(b * S, (b + 1) * S)
        cs_cc[0::2, sl] = cos
        cs_cc[1::2, sl] = cos
        cs_ss[0::2, sl] = -sin
        cs_ss[1::2, sl] = sin
    cs_cc = cs_cc.astype(bf16)
    cs_ss = cs_ss.astype(bf16)

    # diagonal-block causal multiplicative masks [128, 4*512]
    p_i = np.arange(128)[:, None]
    c_i = np.arange(CW)[None, :]
    dmask = np.concatenate(
        [(p_i <= c_i - 128 * r).astype(f32) for r in range(4)], axis=1
    ).astype(bf16)

    # Wo grouped even/odd heads, K-padded to 12 x 128 rows
    Wo3 = Wo.reshape(H, DV, D)
    wo_g = []
    for par in (0, 1):
        g = Wo3[par::2].reshape(8 * DV, D)           # [680, D]
        gp = np.zeros((6 * 128, D), f32)
        gp[:680] = g
        wo_g.append(gp)
    wo_tiled = np.concatenate(wo_g, axis=0).reshape(NKT_WO, 128, D).astype(bf16)

    bo_rep = np.broadcast_to(bo.astype(f32), (128, D)).copy()

    in_maps = []
    for c in range(NCORE):
        heads = [2 * c, 2 * c + 1]
        wq_c = np.empty((D, 256), f32)
        wk_c = np.empty((D, 256), f32)
        bqk_c = np.empty((128, 8), f32)
        for hl, hg in enumerate(heads):
            wq_c[:, hl * 128:(hl + 1) * 128] = Wq[:, hg * 128:(hg + 1) * 128][:, perm]
            wk_c[:, hl * 128:(hl + 1) * 128] = Wk[:, hg * 128:(hg + 1) * 128][:, perm]
            bq_p = bq[hg * 128:(hg + 1) * 128][perm]
            bk_p = bk[hg * 128:(hg + 1) * 128][perm]
            sw = np.arange(128) ^ 1
            bqk_c[:, hl] = bq_p
            bqk_c[:, 2 + hl] = bq_p[sw]
            bqk_c[:, 4 + hl] = bk_p
            bqk_c[:, 6 + hl] = bk_p[sw]
        lin = slice(2 * c * DV, 2 * c * DV + 2 * DV)
        gate = slice(H * DV + 2 * c * DV, H * DV + 2 * c * DV + 2 * DV)
        wv_c = np.concatenate([Wv[:, lin], Wv[:, gate]], axis=1)   # [D, 340]
        bv_c = np.concatenate([bv[lin], bv[gate]])[None, :]        # [1, 340]
        m = {
            "xq_t": xq_t, "xk_t": xk_t, "xv_t": xv_t,
            "wq": wq_c.astype(bf16), "wk": wk_c.astype(bf16),
            "wv": wv_c.astype(bf16),
            "bqk": bqk_c, "bv": bv_c.astype(bf16),
            "cs_cc": cs_cc, "cs_ss": cs_ss,
            "wo": wo_tiled, "bo_rep": bo_rep,
        }
        in_maps.append(m)
    return in_maps, dmask


def build(mode, debug=False):
    """mode in {'causal', 'nomask', 'general'}"""
    import concourse.bass as bass
    import concourse.mybir as mybir
    from contextlib import ExitStack

    F32 = mybir.dt.float32
    F32R = mybir.dt.float32r
    BF = mybir.dt.bfloat16
    Exp = mybir.ActivationFunctionType.Exp
    Silu = mybir.ActivationFunctionType.Silu
    ADD = mybir.AluOpType.add
    MULT = mybir.AluOpType.mult

    nc = bass.Bass()
    d = {}
    d["xq_t"] = nc.dram_tensor("xq_t", [D, T], BF, kind="ExternalInput")
    d["xk_t"] = nc.dram_tensor("xk_t", [D, T], BF, kind="ExternalInput")
    d["xv_t"] = nc.dram_tensor("xv_t", [D, T], BF, kind="ExternalInput")
    d["wq"] = nc.dram_tensor("wq", [D, 256], BF, kind="ExternalInput")
    d["wk"] = nc.dram_tensor("wk", [D, 256], BF, kind="ExternalInput")
    d["wv"] = nc.dram_tensor("wv", [D, 340], BF, kind="ExternalInput")
    d["bqk"] = nc.dram_tensor("bqk", [128, 8], F32, kind="ExternalInput")
    d["bv"] = nc.dram_tensor("bv", [1, 340], BF, kind="ExternalInput")
    d["cs_cc"] = nc.dram_tensor("cs_cc", [128, T], BF, kind="ExternalInput")
    d["cs_ss"] = nc.dram_tensor("cs_ss", [128, T], BF, kind="ExternalInput")
    d["wo"] = nc.dram_tensor("wo", [NKT_WO, 128, D], BF, kind="ExternalInput")
    d["bo_rep"] = nc.dram_tensor("bo_rep", [128, D], F32, kind="ExternalInput")
    if mode == "causal":
        d["dmask"] = nc.dram_tensor("dmask", [128, 4 * CW], BF, kind="ExternalInput")
    if mode == "general":
        d["fmask"] = nc.dram_tensor("fmask", [S, S], BF, kind="ExternalInput")
    out_d = nc.dram_tensor("out", [CW, D], F32, kind="ExternalOutput")
    if debug:
        dbg = {
            "dbg_qT": nc.dram_tensor("dbg_qT", [128, HL * T], BF, kind="ExternalOutput"),
            "dbg_kT": nc.dram_tensor("dbg_kT", [128, HL * T], BF, kind="ExternalOutput"),
            "dbg_vg": nc.dram_tensor("dbg_vg", [128, NTT * HL * DVP], BF, kind="ExternalOutput"),
            "dbg_ao": nc.dram_tensor("dbg_ao", [128, NKT_WO * CW], BF, kind="ExternalOutput"),
            "dbg_oT": nc.dram_tensor("dbg_oT", [DVP, CW], F32, kind="ExternalOutput"),
            "dbg_nrm": nc.dram_tensor("dbg_nrm", [DV, CW], BF, kind="ExternalOutput"),
            "dbg_rcp": nc.dram_tensor("dbg_rcp", [1, CW], BF, kind="ExternalOutput"),
        }
    a2a_in = [nc.dram_tensor(f"a2a_in{h}", [NCORE, DV, CW], BF) for h in range(HL)]
    a2a_out = [nc.dram_tensor(f"a2a_out{h}", [NCORE, DV, CW], BF) for h in range(HL)]

    # ---- iteration schedules -------------------------------------------
    if mode == "general":
        iters = [(h, b, qc) for qc in range(4) for b in range(B) for h in range(HL)]
    else:
        iters = [(h, b, qc) for h in range(HL) for b in range(B) for qc in range(4)]

    def nk_of(qc):
        return 4 * qc + 4 if mode == "causal" else 16

    def is_masked(qc, j):
        if mode == "causal":
            return j >= 4 * qc
        return mode == "general"

    # per-tile bookkeeping (global tile index -> cumulative mask count)
    tiles = []           # list of (i2, jj, nk, masked)
    cum_mask = []
    cm = 0
    for i2, (h, b, qc) in enumerate(iters):
        nk = nk_of(qc)
        for jj in range(nk):
            msk = is_masked(qc, jj)
            if msk:
                cm += 1
            tiles.append((i2, jj, nk, msk))
            cum_mask.append(cm)
    NTILE = len(tiles)
    # first global tile index of each iteration, and last
    iter_first = {}
    iter_last = {}
    for gj, (i2, jj, nk, msk) in enumerate(tiles):
        if jj == 0:
            iter_first[i2] = gj
        if jj == nk - 1:
            iter_last[i2] = gj
    NIT = len(iters)

    # preload order -> s_pre thresholds
    pre_order = ["wq", "wk", "wv", "bqk", "bv", "cs_cc", "cs_ss", "bo_rep"]
    if mode == "causal":
        pre_order.append("dmask")
    PRE = {name: 16 * (i + 1) for i, name in enumerate(pre_order)}

    with ExitStack() as es:
        def sb(name, shape, dt_):
            return es.enter_context(nc.sbuf_tensor(name, shape, dt_))
        wq_s = sb("wq_s", [128, KT * 256], BF)
        wk_s = sb("wk_s", [128, KT * 256], BF)
        wv_s = sb("wv_s", [128, KT * 340], BF)
        bqk_s = sb("bqk_s", [128, 8], F32)
        bv_s = sb("bv_s", [1, 340], BF)
        cs_cc_s = sb("cs_cc_s", [128, T], BF)
        cs_ss_s = sb("cs_ss_s", [128, T], BF)
        bo_s = sb("bo_s", [128, D], F32)
        if mode == "causal":
            dmask_s = sb("dmask_s", [128, 4 * CW], BF)
        if mode == "general":
            fm_s = sb("fm_s", [128, KT * CW], BF)
        xs_ = sb("xs_", [128, 2 * KT * CW], BF)   # x chunk double buffer (aliased by wo bufs in phase 4)
        qT_s = sb("qT_s", [128, HL * T], BF)
        kT_s = sb("kT_s", [128, HL * T], BF)
        vg_s = sb("vg_s", [128, NTT * HL * DVP], BF)
        shf_s = sb("shf_s", [128, CW], F32)
        A_s = sb("A_s", [128, CW], F32)
        B_s = sb("B_s", [128, CW], F32)
        pT = [sb(f"pT{i}", [128, CW], BF) for i in range(4)]
        oT_stage = [sb(f"oT_stage{i}", [DV, CW], BF) for i in range(2)]
        oT_f32 = [sb(f"oT_f32_{i}", [DVP, CW], F32) for i in range(2)]
        recip_fr = [sb(f"recip_fr{i}", [1, CW], BF) for i in range(2)]
        den_s = [sb(f"den_s{i}", [1, CW], F32) for i in range(2)]
        ones_bf = sb("ones_bf", [1, 128], BF)
        ao_s = sb("ao_s", [128, NKT_WO * CW], BF)
        out_s = [sb(f"out_s{i}", [128, CW], F32) for i in range(2)]
        # wo double buffer aliases the (dead in phase 4) x-chunk buffer
        wo_buf = xs_

        ps = [es.enter_context(nc.psum_tensor(f"ps{i}", [128, CW], F32)) for i in range(8)]
        # phase1: q/k banks 0-3 keyed (chunk%2, h); v banks 4-7 keyed t%4
        # phase2: sT banks 0/1, oT banks 2/3, bcast banks 4/5
        # phase4: out banks 6/7

        sem_names = [
            "s_pre", "s_x", "s_qk", "s_rope", "s_v", "s_silu", "s_glu",
            "s_sT", "s_exp", "s_mask", "s_pv", "s_ocp", "s_den", "s_recip", "s_bcast",
            "s_norm", "s_oT", "s_cc", "s_ao", "s_wo_pe", "s_evac",
            "s_out", "s_misc",
        ]
        if mode == "general":
            sem_names.append("s_fm")
        sm = {n: es.enter_context(nc.semaphore(n)) for n in sem_names}
        block = es.enter_context(nc.Block())

        x_stream = [("xq_t", c) for c in range(NCHUNK)] + \
                   [("xk_t", c) for c in range(NCHUNK)] + \
                   [("xv_t", c) for c in range(NCHUNK)]

        # ================= SYNC: DMA engine =================
        @block.sync
        def _(sync):
            for name in pre_order:
                dst = {"wq": wq_s, "wk": wk_s, "wv": wv_s, "bqk": bqk_s,
                       "bv": bv_s, "cs_cc": cs_cc_s, "cs_ss": cs_ss_s,
                       "bo_rep": bo_s}.get(name)
                if name == "dmask":
                    dst = dmask_s
                src = d[name]
                if name in ("wq", "wk", "wv"):
                    w = 256 if name != "wv" else 340
                    sync.dma_start(
                        out=dst[:].rearrange("p (kt w) -> p kt w", kt=KT),
                        in_=src.rearrange("(kt p) w -> p kt w", p=128),
                    ).then_inc(sm["s_pre"], 16)
                else:
                    sync.dma_start(out=dst[:], in_=src[:]).then_inc(sm["s_pre"], 16)

            # x chunk stream
            for gc, (name, c) in enumerate(x_stream):
                buf = gc % 2
                # credit: wait until the previous occupant (chunk gc-2) consumed
                pc = gc - 2
                if pc >= 0:
                    if pc < 16:
                        sync.wait_ge(sm["s_qk"], 2 * pc + 2)
                    else:
                        sync.wait_ge(sm["s_v"], 4 * (pc - 16) + 4)
                sync.dma_start(
                    out=xs_[:, buf * KT * CW:(buf + 1) * KT * CW]
                        .rearrange("p (kt w) -> p kt w", kt=KT),
                    in_=d[name].rearrange("(kt p) t -> p kt t", p=128)
                        [:, :, c * CW:(c + 1) * CW],
                ).then_inc(sm["s_x"], 16)

            # general mode: fmask chunks loaded during phase 2 (qc-outer order)
            if mode == "general":
                for qc in range(4):
                    if qc >= 1:
                        sync.wait_ge(sm["s_mask"], 64 * qc)
                    sync.dma_start(
                        out=fm_s[:].rearrange("p (kt w) -> p kt w", kt=KT),
                        in_=d["fmask"].rearrange("(kt p) t -> p kt t", p=128)
                            [:, :, qc * CW:(qc + 1) * CW],
                    ).then_inc(sm["s_fm"], 16)

            # oT stores to a2a input buffers (+ denominator row bounce
            # to partition 0 -- per-lane engines cannot cross partitions)
            for i2, (h, b, qc) in enumerate(iters):
                t = b * 4 + qc
                sync.wait_ge(sm["s_ocp"], i2 + 1)
                if i2 - 1 >= 1:
                    sync.wait_ge(sm["s_recip"], i2 - 1)
                sync.dma_start(out=den_s[i2 % 2][:], in_=oT_f32[i2 % 2][DEN:DEN + 1, :]
                               ).then_inc(sm["s_den"], 16)
                sync.wait_ge(sm["s_norm"], i2 + 1)
                sync.dma_start(out=a2a_in[h][t, :, :], in_=oT_stage[i2 % 2][:]
                               ).then_inc(sm["s_oT"], 16)

            # phase 4 loads: wo n=0,1 can start now (buffer aliased on xs_)
            for n in (0, 1):
                sync.dma_start(
                    out=wo_buf[:, n * NKT_WO * CW:(n + 1) * NKT_WO * CW]
                        .rearrange("p (kt w) -> p kt w", kt=NKT_WO),
                    in_=d["wo"][:, :, n * CW:(n + 1) * CW]
                        .rearrange("kt p w -> p kt w"),
                ).then_inc(sm["s_ao"], 16)
            # ao loads after collectives
            for g in range(2):
                sync.wait_ge(sm["s_cc"], g + 1)
                flat = a2a_out[g].rearrange("r p t -> (r p) t")
                sync.dma_start(
                    out=ao_s[:, (g * 6) * CW:(g * 6 + 5) * CW]
                        .rearrange("p (kt w) -> p kt w", kt=5),
                    in_=flat[0:640].rearrange("(kt p) t -> p kt t", p=128),
                ).then_inc(sm["s_ao"], 16)
                sync.dma_start(
                    out=ao_s[0:40, (g * 6 + 5) * CW:(g * 6 + 6) * CW],
                    in_=flat[640:680],
                ).then_inc(sm["s_ao"], 16)
            # wo n=2 (after PE consumed n=0), out stores 0-3, wo n=3, rest
            sync.wait_ge(sm["s_wo_pe"], 4)
            sync.dma_start(
                out=wo_buf[:, 0:NKT_WO * CW]
                    .rearrange("p (kt w) -> p kt w", kt=NKT_WO),
                in_=d["wo"][:, :, 2 * CW:3 * CW].rearrange("kt p w -> p kt w"),
            ).then_inc(sm["s_ao"], 16)
            for idx in range(4):
                n, m = idx // 4, idx % 4
                sync.wait_ge(sm["s_evac"], idx + 1)
                sync.dma_start(out=out_d[m * 128:(m + 1) * 128, n * CW:(n + 1) * CW],
                               in_=out_s[idx % 2][:]).then_inc(sm["s_out"], 16)
            sync.wait_ge(sm["s_wo_pe"], 8)
            sync.dma_start(
                out=wo_buf[:, NKT_WO * CW:2 * NKT_WO * CW]
                    .rearrange("p (kt w) -> p kt w", kt=NKT_WO),
                in_=d["wo"][:, :, 3 * CW:4 * CW].rearrange("kt p w -> p kt w"),
            ).then_inc(sm["s_ao"], 16)
            for idx in range(4, 16):
                n, m = idx // 4, idx % 4
                sync.wait_ge(sm["s_evac"], idx + 1)
                sync.dma_start(out=out_d[m * 128:(m + 1) * 128, n * CW:(n + 1) * CW],
                               in_=out_s[idx % 2][:]).then_inc(sm["s_out"], 16)
            sync.wait_ge(sm["s_out"], 16 * 16)
            if debug:
                sync.dma_start(out=dbg["dbg_qT"][:], in_=qT_s[:]).then_inc(sm["s_out"], 16)
                sync.dma_start(out=dbg["dbg_kT"][:], in_=kT_s[:]).then_inc(sm["s_out"], 16)
                sync.dma_start(out=dbg["dbg_vg"][:], in_=vg_s[:]).then_inc(sm["s_out"], 16)
                sync.dma_start(out=dbg["dbg_ao"][:], in_=ao_s[:]).then_inc(sm["s_out"], 16)
                sync.dma_start(out=dbg["dbg_oT"][:], in_=oT_f32[(NIT - 1) % 2][:]).then_inc(sm["s_out"], 16)
                li = (NIT - 1) % 2
                sync.dma_start(out=dbg["dbg_nrm"][:], in_=oT_stage[li][:]).then_inc(sm["s_out"], 16)
                sync.dma_start(out=dbg["dbg_rcp"][:], in_=recip_fr[li][:]).then_inc(sm["s_out"], 16)
                sync.wait_ge(sm["s_out"], 16 * 23)

        # ================= TENSOR: PE =================
        @block.tensor
        def _(tensor):
            mm = nc.tensor.matmul

            def bcast_emit(i):
                tensor.wait_ge(sm["s_recip"], i + 1)
                if i - 1 >= 1:
                    tensor.wait_ge(sm["s_norm"], i - 1)
                mm(ps[4 + i % 2][0:DV, :], ones_bf[0:1, 0:DV],
                   recip_fr[i % 2][:], start=True, stop=True
                   ).then_inc(sm["s_bcast"], 1)

            # ---- phase 1: q/k transposed projections ----
            for tens_i, (xname, wsb, pre_need) in enumerate(
                    [("q", wq_s, PRE["wq"]), ("k", wk_s, PRE["wk"])]):
                for c in range(NCHUNK):
                    gc = tens_i * NCHUNK + c
                    for h in range(HL):
                        gidx = gc * HL + h
                        if gidx == 0:
                            tensor.wait_ge(sm["s_pre"], pre_need)
                        elif h == 0 and c == 0:
                            tensor.wait_ge(sm["s_pre"], pre_need)
                        if h == 0:
                            tensor.wait_ge(sm["s_x"], 16 * (gc + 1))
                        if gidx - 3 >= 1:
                            tensor.wait_ge(sm["s_rope"], gidx - 3)
                        bank = ps[(gc % 2) * 2 + h]
                        buf = gc % 2
                        for kt in range(KT):
                            mm(bank[:],
                               wsb[:, kt * 256 + h * 128: kt * 256 + (h + 1) * 128],
                               xs_[:, buf * KT * CW + kt * CW: buf * KT * CW + (kt + 1) * CW],
                               start=(kt == 0), stop=(kt == KT - 1)
                               ).then_maybe_inc((sm["s_qk"], 1) if kt == KT - 1 else None)

            # ---- phase 1: v projection + bias ----
            tensor.wait_ge(sm["s_pre"], PRE["bv"])
            tensor.wait_ge(sm["s_misc"], 1)
            for t in range(NTT):
                gc = 16 + t // 4
                if t % 4 == 0:
                    tensor.wait_ge(sm["s_x"], 16 * (gc + 1))
                if t - 3 >= 1:
                    tensor.wait_ge(sm["s_glu"], t - 3)
                bank = ps[4 + t % 4]
                buf = gc % 2
                toff = buf * KT * CW + (t % 4) * 128
                for kt in range(KT):
                    mm(bank[:, 0:340],
                       xs_[:, toff + kt * CW: toff + kt * CW + 128],
                       wv_s[:, kt * 340:(kt + 1) * 340],
                       start=(kt == 0), stop=False)
                mm(bank[:, 0:340], ones_bf[0:1, 0:128], bv_s[:],
                   start=False, stop=True).then_inc(sm["s_v"], 1)

            # ---- phase 2: attention ----
            tensor.wait_ge(sm["s_rope"], 32)
            tensor.wait_ge(sm["s_glu"], 32)
            for i2, (h, b, qc) in enumerate(iters):
                nk = nk_of(qc)
                qoff = h * T + b * S + qc * CW
                for jj in range(nk):
                    gj = iter_first[i2] + jj
                    # sT matmul
                    if gj - 1 >= 1:
                        tensor.wait_ge(sm["s_exp"], gj - 1)
                    mm(ps[gj % 2][:],
                       kT_s[:, h * T + b * S + jj * 128: h * T + b * S + (jj + 1) * 128],
                       qT_s[:, qoff: qoff + CW],
                       start=True, stop=True).then_inc(sm["s_sT"], 1)
                    if jj == 0 and i2 >= 1:
                        bcast_emit(i2 - 1)
                    # PV matmul
                    if tiles[gj][3]:
                        tensor.wait_ge(sm["s_mask"], cum_mask[gj])
                    else:
                        tensor.wait_ge(sm["s_exp"], gj + 1)
                    if jj == 0 and i2 - 1 >= 1:
                        tensor.wait_ge(sm["s_ocp"], i2 - 1)
                    g = b * 16 + jj
                    mm(ps[2 + i2 % 2][0:DVP, :],
                       vg_s[:, (g * HL + h) * DVP: (g * HL + h) * DVP + DVP],
                       pT[gj % 4][:],
                       start=(jj == 0), stop=(jj == nk - 1)
                       ).then_inc(sm["s_pv"], 1)
            bcast_emit(NIT - 1)

            # ---- phase 4: Wo ----
            for idx in range(16):
                n, m = idx // 4, idx % 4
                need = {0: 96, 1: 96, 2: 112, 3: 128}[n]
                if m == 0:
                    tensor.wait_ge(sm["s_ao"], need)
                if idx - 1 >= 1:
                    tensor.wait_ge(sm["s_evac"], idx - 1)
                nb = n % 2
                for kt in range(NKT_WO):
                    mm(ps[6 + idx % 2][:],
                       ao_s[:, kt * CW + m * 128: kt * CW + (m + 1) * 128],
                       wo_buf[:, nb * NKT_WO * CW + kt * CW: nb * NKT_WO * CW + (kt + 1) * CW],
                       start=(kt == 0), stop=(kt == NKT_WO - 1)
                       ).then_maybe_inc((sm["s_wo_pe"], 1) if kt == NKT_WO - 1 else None)

        # ================= SCALAR: ACT =================
        @block.scalar
        def _(scalar):
            act = nc.scalar.activation

            def ocp_emit(i):
                # evacuate oT'+denom PSUM -> SBUF f32 (frees oT bank, enables
                # single-PSUM-operand DVE ops downstream)
                scalar.wait_ge(sm["s_pv"], iter_last[i] + 1)
                if i - 1 >= 1:
                    scalar.wait_ge(sm["s_norm"], i - 1)
                nc.scalar.copy(oT_f32[i % 2][:], ps[2 + i % 2][0:DVP, :]
                               ).then_inc(sm["s_ocp"], 1)

            # phase 1: silu for GLU
            for t in range(NTT):
                scalar.wait_ge(sm["s_v"], t + 1)
                if t - 1 >= 1:
                    scalar.wait_ge(sm["s_glu"], t - 1)
                act(pT[t % 2][:, 0:170], ps[4 + t % 4][:, 170:340], Silu
                    ).then_inc(sm["s_silu"], 1)
            # phase 2: exp (+ oT evacuation copies interleaved)
            first_of_iter = {g: i for i, g in iter_first.items()}
            for gj in range(NTILE):
                scalar.wait_ge(sm["s_sT"], gj + 1)
                if gj - 3 >= 1:
                    scalar.wait_ge(sm["s_pv"], gj - 3)
                act(pT[gj % 4][:], ps[gj % 2][:], Exp, scale=SCALE
                    ).then_inc(sm["s_exp"], 1)
                i2n = first_of_iter.get(gj)
                if i2n is not None and i2n >= 1:
                    ocp_emit(i2n - 1)
            ocp_emit(NIT - 1)

        # ================= VECTOR: DVE =================
        @block.vector
        def _(vector):
            vec = nc.vector
            # init constants
            vec.memset(vg_s[:], 1.0)
            vec.memset(ones_bf[:], 1.0)
            vec.memset(ao_s[:, 5 * CW:6 * CW], 0.0)
            vec.memset(ao_s[:, 11 * CW:12 * CW], 0.0).then_inc(sm["s_misc"], 1)

            # phase 1: rope for q then k
            vector.wait_ge(sm["s_pre"], PRE["cs_ss"])
            for gidx in range(32):
                tens_i, rem = divmod(gidx, 16)
                c, h = divmod(rem, HL)
                gc = tens_i * NCHUNK + c
                bank = ps[(gc % 2) * 2 + h]
                dst = (qT_s if tens_i == 0 else kT_s)
                toff = c * CW
                bcol = tens_i * 4 + h
                vector.wait_ge(sm["s_qk"], gidx + 1)
                vec.stream_shuffle(shf_s[:], bank[:], SWAP_MASK)
                vec.scalar_tensor_tensor(
                    A_s[:], bank[:], bqk_s[:, bcol:bcol + 1],
                    cs_cc_s[:, toff:toff + CW], op0=ADD, op1=MULT)
                vec.scalar_tensor_tensor(
                    B_s[:], shf_s[:], bqk_s[:, bcol + 2:bcol + 3],
                    cs_ss_s[:, toff:toff + CW], op0=ADD, op1=MULT)
                vec.tensor_add(dst[:, h * T + toff: h * T + toff + CW],
                               A_s[:], B_s[:]).then_inc(sm["s_rope"], 1)

            # phase 1: GLU muls
            for t in range(NTT):
                vector.wait_ge(sm["s_silu"], t + 1)
                for h in range(HL):
                    ins = vec.tensor_mul(
                        vg_s[:, (t * HL + h) * DVP:(t * HL + h) * DVP + DV],
                        ps[4 + t % 4][:, h * DV:(h + 1) * DV],
                        pT[t % 2][:, h * DV:(h + 1) * DV])
                    if h == HL - 1:
                        ins.then_inc(sm["s_glu"], 1)

            # phase 2: masks / recip / norm
            def norm_emit(i):
                vector.wait_ge(sm["s_bcast"], i + 1)
                if 16 * (i - 1) >= 16:
                    vector.wait_ge(sm["s_oT"], 16 * (i - 1))
                vec.tensor_mul(oT_stage[i % 2][:], oT_f32[i % 2][0:DV, :],
                               ps[4 + i % 2][0:DV, :]).then_inc(sm["s_norm"], 1)

            for i2, (h, b, qc) in enumerate(iters):
                nk = nk_of(qc)
                for jj in range(nk):
                    gj = iter_first[i2] + jj
                    if tiles[gj][3]:
                        vector.wait_ge(sm["s_exp"], gj + 1)
                        if mode == "causal":
                            r = jj - 4 * qc
                            msrc = dmask_s[:, r * CW:(r + 1) * CW]
                        else:
                            vector.wait_ge(sm["s_fm"], 16 * (qc + 1))
                            msrc = fm_s[:, jj * CW:(jj + 1) * CW]
                        vec.tensor_mul(pT[gj % 4][:], pT[gj % 4][:], msrc
                                       ).then_inc(sm["s_mask"], 1)
                # norm for the previous iteration must precede this recip:
                # recip depends on sync's den-bounce DMA, and sync only
                # reaches it after storing iter i2-1 (which needs norm(i2-1)).
                if i2 >= 1:
                    norm_emit(i2 - 1)
                # recip for this iteration (reads the DMA-bounced denom row)
                vector.wait_ge(sm["s_den"], 16 * (i2 + 1))
                if i2 - 1 >= 1:
                    vector.wait_ge(sm["s_bcast"], i2 - 1)
                with nc.allow_low_precision(reason="softmax denom recip in bf16"):
                    vec.reciprocal(recip_fr[i2 % 2][:], den_s[i2 % 2][:]
                                   ).then_inc(sm["s_recip"], 1)
            norm_emit(NIT - 1)

            # phase 4: evac + bias
            vector.wait_ge(sm["s_pre"], PRE["bo_rep"])
            for idx in range(16):
                n = idx // 4
                vector.wait_ge(sm["s_wo_pe"], idx + 1)
                if 16 * (idx - 1) >= 16:
                    vector.wait_ge(sm["s_out"], 16 * (idx - 1))
                vec.tensor_add(out_s[idx % 2][:], ps[6 + idx % 2][:],
                               bo_s[:, n * CW:(n + 1) * CW]).then_inc(sm["s_evac"], 1)

        # ================= GPSIMD: collectives =================
        @block.gpsimd
        def _(gpsimd):
            rg = [list(range(NCORE))]
            for g in range(2):
                gpsimd.wait_ge(sm["s_oT"], 16 * 8 * (g + 1) if mode != "general" else 16 * 16)
                gpsimd.collective_compute(
                    "AllToAll", mybir.AluOpType.bypass,
                    replica_groups=rg,
                    ins=[a2a_in[g][:]], outs=[a2a_out[g][:]],
                ).then_inc(sm["s_cc"], 1)

    return nc


_cache = {}


def _get_nc(mode):
    if mode not in _cache:
        _cache[mode] = build(mode)
    return _cache[mode]


def kernel(q, k, v, mask, Wq, bq, Wk, bk, Wv, bv, Wo, bo):
    from concourse.bass_utils import run_bass_kernel_spmd

    q = np.asarray(q, np.float32)
    k = np.asarray(k, np.float32)
    v = np.asarray(v, np.float32)
    mask = np.asarray(mask)
    Wq = np.asarray(Wq, np.float32); bq = np.asarray(bq, np.float32)
    Wk = np.asarray(Wk, np.float32); bk = np.asarray(bk, np.float32)
    Wv = np.asarray(Wv, np.float32); bv = np.asarray(bv, np.float32)
    Wo = np.asarray(Wo, np.float32); bo = np.asarray(bo, np.float32)

    causal_ref = np.triu(np.ones((S, S), bool), 1)
    if mask.shape == (S, S) and (mask == causal_ref).all():
        mode = "causal"
    elif not mask.any():
        mode = "nomask"
    else:
        mode = "general"

    in_maps, dmask = _host_inputs(q, k, v, mask, Wq, bq, Wk, bk, Wv, bv, Wo, bo)
    if mode == "causal":
        for m in in_maps:
            m["dmask"] = dmask
    elif mode == "general":
        fm = np.where(mask, 0.0, 1.0).T.astype(bf16)   # [k, q] multiplicative
        fm = np.ascontiguousarray(fm)
        for m in in_maps:
            m["fmask"] = fm

    nc = _get_nc(mode)
    tmpdir = os.environ.get("BASS_TMPDIR")
    if tmpdir:
        os.makedirs(tmpdir, exist_ok=True)
    res = run_bass_kernel_spmd(nc, in_maps, list(range(NCORE)), tmpdir=tmpdir)
    LAST["exec_time_ns"] = res.exec_time_ns
    LAST["results"] = res
    out = np.concatenate([res.results[c]["out"] for c in range(NCORE)], axis=0)
    return np.ascontiguousarray(out.reshape(B, S, D).astype(np.float32))


# revision 27
# speedup vs baseline: 1.4612x; 1.4612x over previous
"""Trainium2 Bass kernel for nn_MultiHeadAttention_4690104287263.

Strategy (8 NeuronCores, SPMD):
  - Tensor-parallel over heads: each core owns 2 of the 16 heads.
  - Projections computed TRANSPOSED (qT/kT: [dqk, tok]) so the attention
    matmuls need no on-device transposes. RoPE pairs are interleaved into
    adjacent partitions (baked into the host-side weight column order) so
    the half-rotation becomes a DVE stream_shuffle (even/odd lane swap).
  - Attention computed transposed: sT[k, q] = kT.T @ qT, softmax over the
    partition (k) dim using exp (no max subtraction; scores are O(1)) and
    a ones-column appended to V so the PV matmul also yields the softmax
    denominator for free. Block-causal: upper k-tiles skipped, diagonal
    tiles masked multiplicatively after exp.
  - AllToAll (2 calls, one per local head) reshards attention output from
    head-sharded to token-sharded; Wo is then computed token-sharded with
    the full (zero-padded, even/odd-head-grouped) Wo. No all-reduce.
  - bf16 storage/matmuls with f32 PSUM accumulation throughout.
"""

import os
import numpy as np
import ml_dtypes

bf16 = ml_dtypes.bfloat16

B, S, D, H, DQK, DV = 2, 2048, 2048, 16, 128, 85
T = B * S                 # 4096 flat tokens
NCORE = 8
HL = 2                    # heads per core
SCALE = float(DQK) ** -0.5
ROPE_BASE = 10000.0
CW = 512                  # token chunk width
NCHUNK = T // CW          # 8
KT = D // 128             # 16 k-tiles over d_model
NTT = T // 128            # 32 token tiles
DVP = 97                  # dv(85) + pad + ones column at DEN (32-aligned)
DEN = 96                  # denominator row (must be a legal partition start)
NKT_WO = 12               # Wo K-tiles (2 groups x 6, zero padded 680->768)

SWAP_MASK = [i ^ 1 for i in range(32)]

LAST = {"exec_time_ns": None, "results": None}


def _host_inputs(q, k, v, mask, Wq, bq, Wk, bk, Wv, bv, Wo, bo):
    """Prepare per-core input maps (host-side shard/layout/cast)."""
    f32 = np.float32
    # x^T layouts [D, T], bf16
    xq_t = np.ascontiguousarray(q.reshape(T, D).T).astype(bf16)
    xk_t = np.ascontiguousarray(k.reshape(T, D).T).astype(bf16)
    xv_t = np.ascontiguousarray(v.reshape(T, D).T).astype(bf16)

    # rope pair-interleave permutation within each head's 128 cols
    perm = np.empty(128, np.int64)
    perm[0::2] = np.arange(64)
    perm[1::2] = np.arange(64) + 64

    # cos/sin tables in interleaved layout [128, T]
    inv_freq = 1.0 / (ROPE_BASE ** (np.arange(64, dtype=np.float64) / 64.0))
    pos = np.arange(S, dtype=np.float64)
    ang = pos[:, None] * inv_freq[None, :]           # [S, 64]
    cos = np.cos(ang).T                              # [64, S]
    sin = np.sin(ang).T
    cs_cc = np.empty((128, T), f32)
    cs_ss = np.empty((128, T), f32)
    for b in range(B):
        sl = slice(b * S, (b + 1) * S)
        cs_cc[0::2, sl] = cos
        cs_cc[1::2, sl] = cos
        cs_ss[0::2, sl] = -sin
        cs_ss[1::2, sl] = sin
    cs_cc = cs_cc.astype(bf16)
    cs_ss = cs_ss.astype(bf16)

    # diagonal-block causal multiplicative masks [128, 4*512]
    p_i = np.arange(128)[:, None]
    c_i = np.arange(CW)[None, :]
    dmask = np.concatenate(
        [(p_i <= c_i - 128 * r).astype(f32) for r in range(4)], axis=1
    ).astype(bf16)

    # Wo grouped even/odd heads, K-padded to 12 x 128 rows
    Wo3 = Wo.reshape(H, DV, D)
    wo_g = []
    for par in (0, 1):
        g = Wo3[par::2].reshape(8 * DV, D)           # [680, D]
        gp = np.zeros((6 * 128, D), f32)
        gp[:680] = g
        wo_g.append(gp)
    wo_tiled = np.concatenate(wo_g, axis=0).reshape(NKT_WO, 128, D).astype(bf16)

    bo_rep = np.broadcast_to(bo.astype(f32), (128, D)).copy()

    in_maps = []
    for c in range(NCORE):
        heads = [2 * c, 2 * c + 1]
        wq_c = np.empty((D, 256), f32)
        wk_c = np.empty((D, 256), f32)
        bqk_c = np.empty((128, 8), f32)
        for hl, hg in enumerate(heads):
            wq_c[:, hl * 128:(hl + 1) * 128] = Wq[:, hg * 128:(hg + 1) * 128][:, perm]
            wk_c[:, hl * 128:(hl + 1) * 128] = Wk[:, hg * 128:(hg + 1) * 128][:, perm]
            bq_p = bq[hg * 128:(hg + 1) * 128][perm]
            bk_p = bk[hg * 128:(hg + 1) * 128][perm]
            sw = np.arange(128) ^ 1
            bqk_c[:, hl] = bq_p
            bqk_c[:, 2 + hl] = bq_p[sw]
            bqk_c[:, 4 + hl] = bk_p
            bqk_c[:, 6 + hl] = bk_p[sw]
        lin = slice(2 * c * DV, 2 * c * DV + 2 * DV)
        gate = slice(H * DV + 2 * c * DV, H * DV + 2 * c * DV + 2 * DV)
        wv_c = np.concatenate([Wv[:, lin], Wv[:, gate]], axis=1)   # [D, 340]
        bv_c = np.concatenate([bv[lin], bv[gate]])[None, :]        # [1, 340]
        m = {
            "xq_t": xq_t, "xk_t": xk_t, "xv_t": xv_t,
            "wq": wq_c.astype(bf16), "wk": wk_c.astype(bf16),
            "wv": wv_c.astype(bf16),
            "bqk": bqk_c, "bv": bv_c.astype(bf16),
            "cs_cc": cs_cc, "cs_ss": cs_ss,
            "wo": wo_tiled, "bo_rep": bo_rep,
        }
        in_maps.append(m)
    return in_maps, dmask


def build(mode, debug=False):
    """mode in {'causal', 'nomask', 'general'}"""
    import concourse.bass as bass
    import concourse.mybir as mybir
    from contextlib import ExitStack

    F32 = mybir.dt.float32
    F32R = mybir.dt.float32r
    BF = mybir.dt.bfloat16
    Exp = mybir.ActivationFunctionType.Exp
    Silu = mybir.ActivationFunctionType.Silu
    ADD = mybir.AluOpType.add
    MULT = mybir.AluOpType.mult

    nc = bass.Bass()
    d = {}
    d["xq_t"] = nc.dram_tensor("xq_t", [D, T], BF, kind="ExternalInput")
    d["xk_t"] = nc.dram_tensor("xk_t", [D, T], BF, kind="ExternalInput")
    d["xv_t"] = nc.dram_tensor("xv_t", [D, T], BF, kind="ExternalInput")
    d["wq"] = nc.dram_tensor("wq", [D, 256], BF, kind="ExternalInput")
    d["wk"] = nc.dram_tensor("wk", [D, 256], BF, kind="ExternalInput")
    d["wv"] = nc.dram_tensor("wv", [D, 340], BF, kind="ExternalInput")
    d["bqk"] = nc.dram_tensor("bqk", [128, 8], F32, kind="ExternalInput")
    d["bv"] = nc.dram_tensor("bv", [1, 340], BF, kind="ExternalInput")
    d["cs_cc"] = nc.dram_tensor("cs_cc", [128, T], BF, kind="ExternalInput")
    d["cs_ss"] = nc.dram_tensor("cs_ss", [128, T], BF, kind="ExternalInput")
    d["wo"] = nc.dram_tensor("wo", [NKT_WO, 128, D], BF, kind="ExternalInput")
    d["bo_rep"] = nc.dram_tensor("bo_rep", [128, D], F32, kind="ExternalInput")
    if mode == "causal":
        d["dmask"] = nc.dram_tensor("dmask", [128, 4 * CW], BF, kind="ExternalInput")
    if mode == "general":
        d["fmask"] = nc.dram_tensor("fmask", [S, S], BF, kind="ExternalInput")
    out_d = nc.dram_tensor("out", [CW, D], F32, kind="ExternalOutput")
    if debug:
        dbg = {
            "dbg_qT": nc.dram_tensor("dbg_qT", [128, HL * T], BF, kind="ExternalOutput"),
            "dbg_kT": nc.dram_tensor("dbg_kT", [128, HL * T], BF, kind="ExternalOutput"),
            "dbg_vg": nc.dram_tensor("dbg_vg", [128, NTT * HL * DVP], BF, kind="ExternalOutput"),
            "dbg_ao": nc.dram_tensor("dbg_ao", [128, NKT_WO * CW], BF, kind="ExternalOutput"),
            "dbg_oT": nc.dram_tensor("dbg_oT", [DVP, CW], F32, kind="ExternalOutput"),
            "dbg_nrm": nc.dram_tensor("dbg_nrm", [DV, CW], BF, kind="ExternalOutput"),
            "dbg_rcp": nc.dram_tensor("dbg_rcp", [1, CW], BF, kind="ExternalOutput"),
        }
    a2a_in = [nc.dram_tensor(f"a2a_in{h}", [NCORE, DV, CW], BF) for h in range(HL)]
    a2a_out = [nc.dram_tensor(f"a2a_out{h}", [NCORE, DV, CW], BF) for h in range(HL)]

    # ---- iteration schedules -------------------------------------------
    if mode == "general":
        iters = [(h, b, qc) for qc in range(4) for b in range(B) for h in range(HL)]
    else:
        iters = [(h, b, qc) for h in range(HL) for b in range(B) for qc in range(4)]

    def nk_of(qc):
        return 4 * qc + 4 if mode == "causal" else 16

    def is_masked(qc, j):
        if mode == "causal":
            return j >= 4 * qc
        return mode == "general"

    # per-tile bookkeeping (global tile index -> cumulative mask count)
    tiles = []           # list of (i2, jj, nk, masked)
    cum_mask = []
    cm = 0
    for i2, (h, b, qc) in enumerate(iters):
        nk = nk_of(qc)
        for jj in range(nk):
            msk = is_masked(qc, jj)
            if msk:
                cm += 1
            tiles.append((i2, jj, nk, msk))
            cum_mask.append(cm)
    NTILE = len(tiles)
    # first global tile index of each iteration, and last
    iter_first = {}
    iter_last = {}
    for gj, (i2, jj, nk, msk) in enumerate(tiles):
        if jj == 0:
            iter_first[i2] = gj
        if jj == nk - 1:
            iter_last[i2] = gj
    NIT = len(iters)

    # preload order -> s_pre thresholds
    pre_order = ["wq", "wk", "wv", "bqk", "bv", "cs_cc", "cs_ss", "bo_rep"]
    if mode == "causal":
        pre_order.append("dmask")
    PRE = {name: 16 * (i + 1) for i, name in enumerate(pre_order)}

    with ExitStack() as es:
        def sb(name, shape, dt_):
            return es.enter_context(nc.sbuf_tensor(name, shape, dt_))
        wq_s = sb("wq_s", [128, KT * 256], BF)
        wk_s = sb("wk_s", [128, KT * 256], BF)
        wv_s = sb("wv_s", [128, KT * 340], BF)
        bqk_s = sb("bqk_s", [128, 8], F32)
        bv_s = sb("bv_s", [1, 340], BF)
        cs_cc_s = sb("cs_cc_s", [128, T], BF)
        cs_ss_s = sb("cs_ss_s", [128, T], BF)
        bo_s = sb("bo_s", [128, D], F32)
        if mode == "causal":
            dmask_s = sb("dmask_s", [128, 4 * CW], BF)
        if mode == "general":
            fm_s = sb("fm_s", [128, KT * CW], BF)
        xs_ = sb("xs_", [128, 2 * KT * CW], BF)   # x chunk double buffer (aliased by wo bufs in phase 4)
        qT_s = sb("qT_s", [128, HL * T], BF)
        kT_s = sb("kT_s", [128, HL * T], BF)
        vg_s = sb("vg_s", [128, NTT * HL * DVP], BF)
        shf_s = sb("shf_s", [128, CW], F32)
        A_s = sb("A_s", [128, CW], F32)
        B_s = sb("B_s", [128, CW], F32)
        pT = [sb(f"pT{i}", [128, CW], BF) for i in range(4)]
        oT_stage = [sb(f"oT_stage{i}", [DV, CW], BF) for i in range(2)]
        oT_f32 = [sb(f"oT_f32_{i}", [DVP, CW], F32) for i in range(2)]
        recip_fr = [sb(f"recip_fr{i}", [1, CW], BF) for i in range(2)]
        den_s = [sb(f"den_s{i}", [1, CW], F32) for i in range(2)]
        ones_bf = sb("ones_bf", [1, 128], BF)
        ao_s = sb("ao_s", [128, NKT_WO * CW], BF)
        out_s = [sb(f"out_s{i}", [128, CW], F32) for i in range(2)]
        # wo double buffer aliases the (dead in phase 4) x-chunk buffer
        wo_buf = xs_

        ps = [es.enter_context(nc.psum_tensor(f"ps{i}", [128, CW], F32)) for i in range(8)]
        # phase1: q/k banks 0-3 keyed (chunk%2, h); v banks 4-7 keyed t%4
        # phase2: sT banks 0/1, oT banks 2/3, bcast banks 4/5
        # phase4: out banks 6/7

        sem_names = [
            "s_pre", "s_x", "s_qk", "s_rope", "s_v", "s_silu", "s_glu",
            "s_sT", "s_exp", "s_mask", "s_pv", "s_ocp", "s_den", "s_recip", "s_bcast",
            "s_norm", "s_oT", "s_cc", "s_ao", "s_wo_pe", "s_evac",
            "s_out", "s_misc",
        ]
        if mode == "general":
            sem_names.append("s_fm")
        sm = {n: es.enter_context(nc.semaphore(n)) for n in sem_names}
        block = es.enter_context(nc.Block())

        x_stream = [("xq_t", c) for c in range(NCHUNK)] + \
                   [("xk_t", c) for c in range(NCHUNK)] + \
                   [("xv_t", c) for c in range(NCHUNK)]

        # ================= SYNC: DMA engine =================
        @block.sync
        def _(sync):
            for name in pre_order:
                dst = {"wq": wq_s, "wk": wk_s, "wv": wv_s, "bqk": bqk_s,
                       "bv": bv_s, "cs_cc": cs_cc_s, "cs_ss": cs_ss_s,
                       "bo_rep": bo_s}.get(name)
                if name == "dmask":
                    dst = dmask_s
                src = d[name]
                if name in ("wq", "wk", "wv"):
                    w = 256 if name != "wv" else 340
                    sync.dma_start(
                        out=dst[:].rearrange("p (kt w) -> p kt w", kt=KT),
                        in_=src.rearrange("(kt p) w -> p kt w", p=128),
                    ).then_inc(sm["s_pre"], 16)
                else:
                    sync.dma_start(out=dst[:], in_=src[:]).then_inc(sm["s_pre"], 16)

            # x chunk stream
            for gc, (name, c) in enumerate(x_stream):
                buf = gc % 2
                # credit: wait until the previous occupant (chunk gc-2) consumed
                pc = gc - 2
                if pc >= 0:
                    if pc < 16:
                        sync.wait_ge(sm["s_qk"], 2 * pc + 2)
                    else:
                        sync.wait_ge(sm["s_v"], 4 * (pc - 16) + 4)
                sync.dma_start(
                    out=xs_[:, buf * KT * CW:(buf + 1) * KT * CW]
                        .rearrange("p (kt w) -> p kt w", kt=KT),
                    in_=d[name].rearrange("(kt p) t -> p kt t", p=128)
                        [:, :, c * CW:(c + 1) * CW],
                ).then_inc(sm["s_x"], 16)

            # general mode: fmask chunks loaded during phase 2 (qc-outer order)
            if mode == "general":
                for qc in range(4):
                    if qc >= 1:
                        sync.wait_ge(sm["s_mask"], 64 * qc)
                    sync.dma_start(
                        out=fm_s[:].rearrange("p (kt w) -> p kt w", kt=KT),
                        in_=d["fmask"].rearrange("(kt p) t -> p kt t", p=128)
                            [:, :, qc * CW:(qc + 1) * CW],
                    ).then_inc(sm["s_fm"], 16)

            # oT stores to a2a input buffers (+ denominator row bounce
            # to partition 0 -- per-lane engines cannot cross partitions)
            for i2, (h, b, qc) in enumerate(iters):
                t = b * 4 + qc
                sync.wait_ge(sm["s_ocp"], i2 + 1)
                if i2 - 1 >= 1:
                    sync.wait_ge(sm["s_recip"], i2 - 1)
                sync.dma_start(out=den_s[i2 % 2][:], in_=oT_f32[i2 % 2][DEN:DEN + 1, :]
                               ).then_inc(sm["s_den"], 16)
                sync.wait_ge(sm["s_norm"], i2 + 1)
                sync.dma_start(out=a2a_in[h][t, :, :], in_=oT_stage[i2 % 2][:]
                               ).then_inc(sm["s_oT"], 16)

            # phase 4 loads: wo n=0,1 can start now (buffer aliased on xs_)
            for n in (0, 1):
                sync.dma_start(
                    out=wo_buf[:, n * NKT_WO * CW:(n + 1) * NKT_WO * CW]
                        .rearrange("p (kt w) -> p kt w", kt=NKT_WO),
                    in_=d["wo"][:, :, n * CW:(n + 1) * CW]
                        .rearrange("kt p w -> p kt w"),
                ).then_inc(sm["s_ao"], 16)
            # ao loads after collectives
            for g in range(2):
                sync.wait_ge(sm["s_cc"], g + 1)
                flat = a2a_out[g].rearrange("r p t -> (r p) t")
                sync.dma_start(
                    out=ao_s[:, (g * 6) * CW:(g * 6 + 5) * CW]
                        .rearrange("p (kt w) -> p kt w", kt=5),
                    in_=flat[0:640].rearrange("(kt p) t -> p kt t", p=128),
                ).then_inc(sm["s_ao"], 16)
                sync.dma_start(
                    out=ao_s[0:40, (g * 6 + 5) * CW:(g * 6 + 6) * CW],
                    in_=flat[640:680],
                ).then_inc(sm["s_ao"], 16)
            # wo n=2 (after PE consumed n=0), out stores 0-3, wo n=3, rest
            sync.wait_ge(sm["s_wo_pe"], 4)
            sync.dma_start(
                out=wo_buf[:, 0:NKT_WO * CW]
                    .rearrange("p (kt w) -> p kt w", kt=NKT_WO),
                in_=d["wo"][:, :, 2 * CW:3 * CW].rearrange("kt p w -> p kt w"),
            ).then_inc(sm["s_ao"], 16)
            for idx in range(4):
                n, m = idx // 4, idx % 4
                sync.wait_ge(sm["s_evac"], idx + 1)
                sync.dma_start(out=out_d[m * 128:(m + 1) * 128, n * CW:(n + 1) * CW],
                               in_=out_s[idx % 2][:]).then_inc(sm["s_out"], 16)
            sync.wait_ge(sm["s_wo_pe"], 8)
            sync.dma_start(
                out=wo_buf[:, NKT_WO * CW:2 * NKT_WO * CW]
                    .rearrange("p (kt w) -> p kt w", kt=NKT_WO),
                in_=d["wo"][:, :, 3 * CW:4 * CW].rearrange("kt p w -> p kt w"),
            ).then_inc(sm["s_ao"], 16)
            for idx in range(4, 16):
                n, m = idx // 4, idx % 4
                sync.wait_ge(sm["s_evac"], idx + 1)
                sync.dma_start(out=out_d[m * 128:(m + 1) * 128, n * CW:(n + 1) * CW],
                               in_=out_s[idx % 2][:]).then_inc(sm["s_out"], 16)
            sync.wait_ge(sm["s_out"], 16 * 16)
            if debug:
                sync.dma_start(out=dbg["dbg_qT"][:], in_=qT_s[:]).then_inc(sm["s_out"], 16)
                sync.dma_start(out=dbg["dbg_kT"][:], in_=kT_s[:]).then_inc(sm["s_out"], 16)
                sync.dma_start(out=dbg["dbg_vg"][:], in_=vg_s[:]).then_inc(sm["s_out"], 16)
                sync.dma_start(out=dbg["dbg_ao"][:], in_=ao_s[:]).then_inc(sm["s_out"], 16)
                sync.dma_start(out=dbg["dbg_oT"][:], in_=oT_f32[(NIT - 1) % 2][:]).then_inc(sm["s_out"], 16)
                li = (NIT - 1) % 2
                sync.dma_start(out=dbg["dbg_nrm"][:], in_=oT_stage[li][:]).then_inc(sm["s_out"], 16)
                sync.dma_start(out=dbg["dbg_rcp"][:], in_=recip_fr[li][:]).then_inc(sm["s_out"], 16)
                sync.wait_ge(sm["s_out"], 16 * 23)

        # ================= TENSOR: PE =================
        @block.tensor
        def _(tensor):
            mm = nc.tensor.matmul

            def bcast_emit(i):
                tensor.wait_ge(sm["s_recip"], i + 1)
                if i >= 1:
                    tensor.wait_ge(sm["s_norm"], i)
                mm(ps[4][0:DV, :], ones_bf[0:1, 0:DV],
                   recip_fr[i % 2][:], start=True, stop=True
                   ).then_inc(sm["s_bcast"], 1)

            # ---- phase 1: q/k transposed projections ----
            for tens_i, (xname, wsb, pre_need) in enumerate(
                    [("q", wq_s, PRE["wq"]), ("k", wk_s, PRE["wk"])]):
                for c in range(NCHUNK):
                    gc = tens_i * NCHUNK + c
                    for h in range(HL):
                        gidx = gc * HL + h
                        if gidx == 0:
                            tensor.wait_ge(sm["s_pre"], pre_need)
                        elif h == 0 and c == 0:
                            tensor.wait_ge(sm["s_pre"], pre_need)
                        if h == 0:
                            tensor.wait_ge(sm["s_x"], 16 * (gc + 1))
                        if gidx - 3 >= 1:
                            tensor.wait_ge(sm["s_rope"], gidx - 3)
                        bank = ps[(gc % 2) * 2 + h]
                        buf = gc % 2
                        for kt in range(KT):
                            mm(bank[:],
                               wsb[:, kt * 256 + h * 128: kt * 256 + (h + 1) * 128],
                               xs_[:, buf * KT * CW + kt * CW: buf * KT * CW + (kt + 1) * CW],
                               start=(kt == 0), stop=(kt == KT - 1)
                               ).then_maybe_inc((sm["s_qk"], 1) if kt == KT - 1 else None)

            # ---- phase 1: v projection + bias ----
            tensor.wait_ge(sm["s_pre"], PRE["bv"])
            tensor.wait_ge(sm["s_misc"], 1)
            for t in range(NTT):
                gc = 16 + t // 4
                if t % 4 == 0:
                    tensor.wait_ge(sm["s_x"], 16 * (gc + 1))
                if t - 3 >= 1:
                    tensor.wait_ge(sm["s_glu"], t - 3)
                bank = ps[4 + t % 4]
                buf = gc % 2
                toff = buf * KT * CW + (t % 4) * 128
                for kt in range(KT):
                    mm(bank[:, 0:340],
                       xs_[:, toff + kt * CW: toff + kt * CW + 128],
                       wv_s[:, kt * 340:(kt + 1) * 340],
                       start=(kt == 0), stop=False)
                mm(bank[:, 0:340], ones_bf[0:1, 0:128], bv_s[:],
                   start=False, stop=True).then_inc(sm["s_v"], 1)

            # ---- phase 2: attention (software-pipelined, PV lags sT by
            # LAG tiles so the exp/mask waits are pre-satisfied) ----
            tensor.wait_ge(sm["s_rope"], 32)
            tensor.wait_ge(sm["s_glu"], 32)
            LAG = 2
            sT_banks = [ps[0], ps[1], ps[5]]

            def emit_sT(gj):
                i2, jj, nk, msk = tiles[gj]
                h, b, qc = iters[i2]
                mm(sT_banks[gj % 3][:],
                   kT_s[:, h * T + b * S + jj * 128: h * T + b * S + (jj + 1) * 128],
                   qT_s[:, h * T + b * S + qc * CW: h * T + b * S + qc * CW + CW],
                   start=True, stop=True).then_inc(sm["s_sT"], 1)

            def emit_pv(gj):
                i2, jj, nk, msk = tiles[gj]
                h, b, qc = iters[i2]
                if msk:
                    tensor.wait_ge(sm["s_mask"], cum_mask[gj])
                else:
                    tensor.wait_ge(sm["s_exp"], gj + 1)
                if jj == 0 and i2 - 1 >= 1:
                    tensor.wait_ge(sm["s_ocp"], i2 - 1)
                g = b * 16 + jj
                mm(ps[2 + i2 % 2][0:DVP, :],
                   vg_s[:, (g * HL + h) * DVP: (g * HL + h) * DVP + DVP],
                   pT[gj % 4][:],
                   start=(jj == 0), stop=(jj == nk - 1)
                   ).then_inc(sm["s_pv"], 1)

            from collections import deque
            pvq = deque()
            bcast_due = {}       # op_idx -> iter index
            opi = 0

            def flush_due():
                nonlocal opi
                for at in sorted(list(bcast_due)):
                    if at <= opi:
                        bcast_emit(bcast_due.pop(at))

            for gj in range(NTILE):
                flush_due()
                emit_sT(gj); opi += 1
                pvq.append(gj)
                if len(pvq) > LAG:
                    flush_due()
                    g2 = pvq.popleft()
                    emit_pv(g2); opi += 1
                    if g2 == iter_last[tiles[g2][0]]:
                        bcast_due[opi + 3] = tiles[g2][0]
            while pvq:
                flush_due()
                g2 = pvq.popleft()
                emit_pv(g2); opi += 1
                if g2 == iter_last[tiles[g2][0]]:
                    bcast_due[opi + 3] = tiles[g2][0]
            for at in sorted(list(bcast_due)):
                bcast_emit(bcast_due.pop(at))

            # ---- phase 4: Wo ----
            for idx in range(16):
                n, m = idx // 4, idx % 4
                need = {0: 96, 1: 96, 2: 112, 3: 128}[n]
                if m == 0:
                    tensor.wait_ge(sm["s_ao"], need)
                if idx - 1 >= 1:
                    tensor.wait_ge(sm["s_evac"], idx - 1)
                nb = n % 2
                for kt in range(NKT_WO):
                    mm(ps[6 + idx % 2][:],
                       ao_s[:, kt * CW + m * 128: kt * CW + (m + 1) * 128],
                       wo_buf[:, nb * NKT_WO * CW + kt * CW: nb * NKT_WO * CW + (kt + 1) * CW],
                       start=(kt == 0), stop=(kt == NKT_WO - 1)
                       ).then_maybe_inc((sm["s_wo_pe"], 1) if kt == NKT_WO - 1 else None)

        # ================= SCALAR: ACT =================
        @block.scalar
        def _(scalar):
            act = nc.scalar.activation

            def ocp_emit(i):
                # evacuate oT'+denom PSUM -> SBUF f32 (frees oT bank, enables
                # single-PSUM-operand DVE ops downstream)
                scalar.wait_ge(sm["s_pv"], iter_last[i] + 1)
                if i - 1 >= 1:
                    scalar.wait_ge(sm["s_norm"], i - 1)
                nc.scalar.copy(oT_f32[i % 2][:], ps[2 + i % 2][0:DVP, :]
                               ).then_inc(sm["s_ocp"], 1)

            # phase 1: silu for GLU
            for t in range(NTT):
                scalar.wait_ge(sm["s_v"], t + 1)
                if t - 1 >= 1:
                    scalar.wait_ge(sm["s_glu"], t - 1)
                act(pT[t % 2][:, 0:170], ps[4 + t % 4][:, 170:340], Silu
                    ).then_inc(sm["s_silu"], 1)
            # phase 2: exp (+ oT evacuation copies; ocp(i) goes after the
            # 2nd exp of iter i+1 so its s_pv wait is pre-satisfied under
            # the PE's lagged PV schedule)
            sT_banks_a = [ps[0], ps[1], ps[5]]
            ocp_after = {}
            for i in range(1, NIT):
                ocp_after[min(iter_first[i] + 1, iter_last[i])] = i - 1
            for gj in range(NTILE):
                scalar.wait_ge(sm["s_sT"], gj + 1)
                if gj - 3 >= 1:
                    scalar.wait_ge(sm["s_pv"], gj - 3)
                act(pT[gj % 4][:], sT_banks_a[gj % 3][:], Exp, scale=SCALE
                    ).then_inc(sm["s_exp"], 1)
                if gj in ocp_after:
                    ocp_emit(ocp_after[gj])
            ocp_emit(NIT - 1)

        # ================= VECTOR: DVE =================
        @block.vector
        def _(vector):
            vec = nc.vector
            # init constants
            vec.memset(vg_s[:], 1.0)
            vec.memset(ones_bf[:], 1.0)
            vec.memset(ao_s[:, 5 * CW:6 * CW], 0.0)
            vec.memset(ao_s[:, 11 * CW:12 * CW], 0.0).then_inc(sm["s_misc"], 1)

            # phase 1: rope for q then k
            vector.wait_ge(sm["s_pre"], PRE["cs_ss"])
            for gidx in range(32):
                tens_i, rem = divmod(gidx, 16)
                c, h = divmod(rem, HL)
                gc = tens_i * NCHUNK + c
                bank = ps[(gc % 2) * 2 + h]
                dst = (qT_s if tens_i == 0 else kT_s)
                toff = c * CW
                bcol = tens_i * 4 + h
                vector.wait_ge(sm["s_qk"], gidx + 1)
                vec.stream_shuffle(shf_s[:], bank[:], SWAP_MASK)
                vec.scalar_tensor_tensor(
                    A_s[:], bank[:], bqk_s[:, bcol:bcol + 1],
                    cs_cc_s[:, toff:toff + CW], op0=ADD, op1=MULT)
                vec.scalar_tensor_tensor(
                    B_s[:], shf_s[:], bqk_s[:, bcol + 2:bcol + 3],
                    cs_ss_s[:, toff:toff + CW], op0=ADD, op1=MULT)
                vec.tensor_add(dst[:, h * T + toff: h * T + toff + CW],
                               A_s[:], B_s[:]).then_inc(sm["s_rope"], 1)

            # phase 1: GLU muls
            for t in range(NTT):
                vector.wait_ge(sm["s_silu"], t + 1)
                for h in range(HL):
                    ins = vec.tensor_mul(
                        vg_s[:, (t * HL + h) * DVP:(t * HL + h) * DVP + DV],
                        ps[4 + t % 4][:, h * DV:(h + 1) * DV],
                        pT[t % 2][:, h * DV:(h + 1) * DV])
                    if h == HL - 1:
                        ins.then_inc(sm["s_glu"], 1)

            # phase 2: masks / recip / norm
            def norm_emit(i):
                vector.wait_ge(sm["s_bcast"], i + 1)
                if 16 * (i - 1) >= 16:
                    vector.wait_ge(sm["s_oT"], 16 * (i - 1))
                vec.tensor_mul(oT_stage[i % 2][:], oT_f32[i % 2][0:DV, :],
                               ps[4][0:DV, :]).then_inc(sm["s_norm"], 1)

            for i2, (h, b, qc) in enumerate(iters):
                nk = nk_of(qc)
                for jj in range(nk):
                    gj = iter_first[i2] + jj
                    if tiles[gj][3]:
                        vector.wait_ge(sm["s_exp"], gj + 1)
                        if mode == "causal":
                            r = jj - 4 * qc
                            msrc = dmask_s[:, r * CW:(r + 1) * CW]
                        else:
                            vector.wait_ge(sm["s_fm"], 16 * (qc + 1))
                            msrc = fm_s[:, jj * CW:(jj + 1) * CW]
                        vec.tensor_mul(pT[gj % 4][:], pT[gj % 4][:], msrc
                                       ).then_inc(sm["s_mask"], 1)
                # norm for the previous iteration must precede this recip:
                # recip depends on sync's den-bounce DMA, and sync only
                # reaches it after storing iter i2-1 (which needs norm(i2-1)).
                if i2 >= 1:
                    norm_emit(i2 - 1)
                # recip for this iteration (reads the DMA-bounced denom row)
                vector.wait_ge(sm["s_den"], 16 * (i2 + 1))
                if i2 - 1 >= 1:
                    vector.wait_ge(sm["s_bcast"], i2 - 1)
                with nc.allow_low_precision(reason="softmax denom recip in bf16"):
                    vec.reciprocal(recip_fr[i2 % 2][:], den_s[i2 % 2][:]
                                   ).then_inc(sm["s_recip"], 1)
            norm_emit(NIT - 1)

            # phase 4: evac + bias
            vector.wait_ge(sm["s_pre"], PRE["bo_rep"])
            for idx in range(16):
                n = idx // 4
                vector.wait_ge(sm["s_wo_pe"], idx + 1)
                if 16 * (idx - 1) >= 16:
                    vector.wait_ge(sm["s_out"], 16 * (idx - 1))
                vec.tensor_add(out_s[idx % 2][:], ps[6 + idx % 2][:],
                               bo_s[:, n * CW:(n + 1) * CW]).then_inc(sm["s_evac"], 1)

        # ================= GPSIMD: collectives =================
        @block.gpsimd
        def _(gpsimd):
            rg = [list(range(NCORE))]
            for g in range(2):
                gpsimd.wait_ge(sm["s_oT"], 16 * 8 * (g + 1) if mode != "general" else 16 * 16)
                gpsimd.collective_compute(
                    "AllToAll", mybir.AluOpType.bypass,
                    replica_groups=rg,
                    ins=[a2a_in[g][:]], outs=[a2a_out[g][:]],
                ).then_inc(sm["s_cc"], 1)

    return nc


_cache = {}


def _get_nc(mode):
    if mode not in _cache:
        _cache[mode] = build(mode)
    return _cache[mode]


def kernel(q, k, v, mask, Wq, bq, Wk, bk, Wv, bv, Wo, bo):
    from concourse.bass_utils import run_bass_kernel_spmd

    q = np.asarray(q, np.float32)
    k = np.asarray(k, np.float32)
    v = np.asarray(v, np.float32)
    mask = np.asarray(mask)
    Wq = np.asarray(Wq, np.float32); bq = np.asarray(bq, np.float32)
    Wk = np.asarray(Wk, np.float32); bk = np.asarray(bk, np.float32)
    Wv = np.asarray(Wv, np.float32); bv = np.asarray(bv, np.float32)
    Wo = np.asarray(Wo, np.float32); bo = np.asarray(bo, np.float32)

    causal_ref = np.triu(np.ones((S, S), bool), 1)
    if mask.shape == (S, S) and (mask == causal_ref).all():
        mode = "causal"
    elif not mask.any():
        mode = "nomask"
    else:
        mode = "general"

    in_maps, dmask = _host_inputs(q, k, v, mask, Wq, bq, Wk, bk, Wv, bv, Wo, bo)
    if mode == "causal":
        for m in in_maps:
            m["dmask"] = dmask
    elif mode == "general":
        fm = np.where(mask, 0.0, 1.0).T.astype(bf16)   # [k, q] multiplicative
        fm = np.ascontiguousarray(fm)
        for m in in_maps:
            m["fmask"] = fm

    nc = _get_nc(mode)
    tmpdir = os.environ.get("BASS_TMPDIR")
    if tmpdir:
        os.makedirs(tmpdir, exist_ok=True)
    res = run_bass_kernel_spmd(nc, in_maps, list(range(NCORE)), tmpdir=tmpdir)
    LAST["exec_time_ns"] = res.exec_time_ns
    LAST["results"] = res
    out = np.concatenate([res.results[c]["out"] for c in range(NCORE)], axis=0)
    return np.ascontiguousarray(out.reshape(B, S, D).astype(np.float32))


# revision 28
# speedup vs baseline: 1.5147x; 1.0366x over previous
"""Trainium2 Bass kernel for nn_MultiHeadAttention_4690104287263.

Strategy (8 NeuronCores, SPMD):
  - Tensor-parallel over heads: each core owns 2 of the 16 heads.
  - Projections computed TRANSPOSED (qT/kT: [dqk, tok]) so the attention
    matmuls need no on-device transposes. RoPE pairs are interleaved into
    adjacent partitions (baked into the host-side weight column order) so
    the half-rotation becomes a DVE stream_shuffle (even/odd lane swap).
  - Attention computed transposed: sT[k, q] = kT.T @ qT, softmax over the
    partition (k) dim using exp (no max subtraction; scores are O(1)) and
    a ones-column appended to V so the PV matmul also yields the softmax
    denominator for free. Block-causal: upper k-tiles skipped, diagonal
    tiles masked multiplicatively after exp.
  - AllToAll (2 calls, one per local head) reshards attention output from
    head-sharded to token-sharded; Wo is then computed token-sharded with
    the full (zero-padded, even/odd-head-grouped) Wo. No all-reduce.
  - bf16 storage/matmuls with f32 PSUM accumulation throughout.
"""

import os
import numpy as np
import ml_dtypes

bf16 = ml_dtypes.bfloat16

B, S, D, H, DQK, DV = 2, 2048, 2048, 16, 128, 85
T = B * S                 # 4096 flat tokens
NCORE = 8
HL = 2                    # heads per core
SCALE = float(DQK) ** -0.5
ROPE_BASE = 10000.0
CW = 512                  # token chunk width
NCHUNK = T // CW          # 8
KT = D // 128             # 16 k-tiles over d_model
NTT = T // 128            # 32 token tiles
DVP = 97                  # dv(85) + pad + ones column at DEN (32-aligned)
DEN = 96                  # denominator row (must be a legal partition start)
NKT_WO = 12               # Wo K-tiles (2 groups x 6, zero padded 680->768)

SWAP_MASK = [i ^ 1 for i in range(32)]

LAST = {"exec_time_ns": None, "results": None}


def _host_inputs(q, k, v, mask, Wq, bq, Wk, bk, Wv, bv, Wo, bo):
    """Prepare per-core input maps (host-side shard/layout/cast)."""
    f32 = np.float32
    # x^T layouts [D, T], bf16
    xq_t = np.ascontiguousarray(q.reshape(T, D).T).astype(bf16)
    xk_t = np.ascontiguousarray(k.reshape(T, D).T).astype(bf16)
    xv_t = np.ascontiguousarray(v.reshape(T, D).T).astype(bf16)

    # rope pair-interleave permutation within each head's 128 cols
    perm = np.empty(128, np.int64)
    perm[0::2] = np.arange(64)
    perm[1::2] = np.arange(64) + 64

    # cos/sin tables in interleaved layout [128, T]
    inv_freq = 1.0 / (ROPE_BASE ** (np.arange(64, dtype=np.float64) / 64.0))
    pos = np.arange(S, dtype=np.float64)
    ang = pos[:, None] * inv_freq[None, :]           # [S, 64]
    cos = np.cos(ang).T                              # [64, S]
    sin = np.sin(ang).T
    cs_cc = np.empty((128, T), f32)
    cs_ss = np.empty((128, T), f32)
    for b in range(B):
        sl = slice(b * S, (b + 1) * S)
        cs_cc[0::2, sl] = cos
        cs_cc[1::2, sl] = cos
        cs_ss[0::2, sl] = -sin
        cs_ss[1::2, sl] = sin
    cs_cc = cs_cc.astype(bf16)
    cs_ss = cs_ss.astype(bf16)

    # diagonal-block causal multiplicative masks [128, 4*512]
    p_i = np.arange(128)[:, None]
    c_i = np.arange(CW)[None, :]
    dmask = np.concatenate(
        [(p_i <= c_i - 128 * r).astype(f32) for r in range(4)], axis=1
    ).astype(bf16)

    # Wo grouped even/odd heads, K-padded to 12 x 128 rows
    Wo3 = Wo.reshape(H, DV, D)
    wo_g = []
    for par in (0, 1):
        g = Wo3[par::2].reshape(8 * DV, D)           # [680, D]
        gp = np.zeros((6 * 128, D), f32)
        gp[:680] = g
        wo_g.append(gp)
    wo_tiled = np.concatenate(wo_g, axis=0).reshape(NKT_WO, 128, D).astype(bf16)

    bo_rep = np.broadcast_to(bo.astype(f32), (128, D)).copy()

    in_maps = []
    for c in range(NCORE):
        heads = [2 * c, 2 * c + 1]
        wq_c = np.empty((D, 256), f32)
        wk_c = np.empty((D, 256), f32)
        bqk_c = np.empty((128, 8), f32)
        for hl, hg in enumerate(heads):
            wq_c[:, hl * 128:(hl + 1) * 128] = Wq[:, hg * 128:(hg + 1) * 128][:, perm]
            wk_c[:, hl * 128:(hl + 1) * 128] = Wk[:, hg * 128:(hg + 1) * 128][:, perm]
            bq_p = bq[hg * 128:(hg + 1) * 128][perm]
            bk_p = bk[hg * 128:(hg + 1) * 128][perm]
            sw = np.arange(128) ^ 1
            bqk_c[:, hl] = bq_p
            bqk_c[:, 2 + hl] = bq_p[sw]
            bqk_c[:, 4 + hl] = bk_p
            bqk_c[:, 6 + hl] = bk_p[sw]
        lin = slice(2 * c * DV, 2 * c * DV + 2 * DV)
        gate = slice(H * DV + 2 * c * DV, H * DV + 2 * c * DV + 2 * DV)
        wv_c = np.concatenate([Wv[:, lin], Wv[:, gate]], axis=1)   # [D, 340]
        bv_c = np.concatenate([bv[lin], bv[gate]])[None, :]        # [1, 340]
        m = {
            "xq_t": xq_t, "xk_t": xk_t, "xv_t": xv_t,
            "wq": wq_c.astype(bf16), "wk": wk_c.astype(bf16),
            "wv": wv_c.astype(bf16),
            "bqk": bqk_c, "bv": bv_c.astype(bf16),
            "cs_cc": cs_cc, "cs_ss": cs_ss,
            "wo": wo_tiled, "bo_rep": bo_rep,
        }
        in_maps.append(m)
    return in_maps, dmask


def build(mode, debug=False):
    """mode in {'causal', 'nomask', 'general'}"""
    import concourse.bass as bass
    import concourse.mybir as mybir
    from contextlib import ExitStack

    F32 = mybir.dt.float32
    F32R = mybir.dt.float32r
    BF = mybir.dt.bfloat16
    Exp = mybir.ActivationFunctionType.Exp
    Silu = mybir.ActivationFunctionType.Silu
    ADD = mybir.AluOpType.add
    MULT = mybir.AluOpType.mult

    nc = bass.Bass()
    d = {}
    d["xq_t"] = nc.dram_tensor("xq_t", [D, T], BF, kind="ExternalInput")
    d["xk_t"] = nc.dram_tensor("xk_t", [D, T], BF, kind="ExternalInput")
    d["xv_t"] = nc.dram_tensor("xv_t", [D, T], BF, kind="ExternalInput")
    d["wq"] = nc.dram_tensor("wq", [D, 256], BF, kind="ExternalInput")
    d["wk"] = nc.dram_tensor("wk", [D, 256], BF, kind="ExternalInput")
    d["wv"] = nc.dram_tensor("wv", [D, 340], BF, kind="ExternalInput")
    d["bqk"] = nc.dram_tensor("bqk", [128, 8], F32, kind="ExternalInput")
    d["bv"] = nc.dram_tensor("bv", [1, 340], BF, kind="ExternalInput")
    d["cs_cc"] = nc.dram_tensor("cs_cc", [128, T], BF, kind="ExternalInput")
    d["cs_ss"] = nc.dram_tensor("cs_ss", [128, T], BF, kind="ExternalInput")
    d["wo"] = nc.dram_tensor("wo", [NKT_WO, 128, D], BF, kind="ExternalInput")
    d["bo_rep"] = nc.dram_tensor("bo_rep", [128, D], F32, kind="ExternalInput")
    if mode == "causal":
        d["dmask"] = nc.dram_tensor("dmask", [128, 4 * CW], BF, kind="ExternalInput")
    if mode == "general":
        d["fmask"] = nc.dram_tensor("fmask", [S, S], BF, kind="ExternalInput")
    out_d = nc.dram_tensor("out", [CW, D], F32, kind="ExternalOutput")
    if debug:
        dbg = {
            "dbg_qT": nc.dram_tensor("dbg_qT", [128, HL * T], BF, kind="ExternalOutput"),
            "dbg_kT": nc.dram_tensor("dbg_kT", [128, HL * T], BF, kind="ExternalOutput"),
            "dbg_vg": nc.dram_tensor("dbg_vg", [128, NTT * HL * DVP], BF, kind="ExternalOutput"),
            "dbg_ao": nc.dram_tensor("dbg_ao", [128, NKT_WO * CW], BF, kind="ExternalOutput"),
            "dbg_oT": nc.dram_tensor("dbg_oT", [DVP, CW], F32, kind="ExternalOutput"),
            "dbg_nrm": nc.dram_tensor("dbg_nrm", [DV, CW], BF, kind="ExternalOutput"),
            "dbg_rcp": nc.dram_tensor("dbg_rcp", [1, CW], BF, kind="ExternalOutput"),
        }
    a2a_in = [nc.dram_tensor(f"a2a_in{h}", [NCORE, DV, CW], BF) for h in range(HL)]
    a2a_out = [nc.dram_tensor(f"a2a_out{h}", [NCORE, DV, CW], BF) for h in range(HL)]

    # ---- iteration schedules -------------------------------------------
    if mode == "general":
        iters = [(h, b, qc) for qc in range(4) for b in range(B) for h in range(HL)]
    else:
        iters = [(h, b, qc) for h in range(HL) for b in range(B) for qc in range(4)]

    def nk_of(qc):
        return 4 * qc + 4 if mode == "causal" else 16

    def is_masked(qc, j):
        if mode == "causal":
            return j >= 4 * qc
        return mode == "general"

    # per-tile bookkeeping (global tile index -> cumulative mask count)
    tiles = []           # list of (i2, jj, nk, masked)
    cum_mask = []
    cm = 0
    for i2, (h, b, qc) in enumerate(iters):
        nk = nk_of(qc)
        for jj in range(nk):
            msk = is_masked(qc, jj)
            if msk:
                cm += 1
            tiles.append((i2, jj, nk, msk))
            cum_mask.append(cm)
    NTILE = len(tiles)
    # first global tile index of each iteration, and last
    iter_first = {}
    iter_last = {}
    for gj, (i2, jj, nk, msk) in enumerate(tiles):
        if jj == 0:
            iter_first[i2] = gj
        if jj == nk - 1:
            iter_last[i2] = gj
    NIT = len(iters)

    # preload order -> s_pre thresholds
    pre_order = ["wq", "wk", "wv", "bqk", "bv", "cs_cc", "cs_ss", "bo_rep"]
    if mode == "causal":
        pre_order.append("dmask")
    PRE = {name: 16 * (i + 1) for i, name in enumerate(pre_order)}

    with ExitStack() as es:
        def sb(name, shape, dt_):
            return es.enter_context(nc.sbuf_tensor(name, shape, dt_))
        wq_s = sb("wq_s", [128, KT * 256], BF)
        wk_s = sb("wk_s", [128, KT * 256], BF)
        wv_s = sb("wv_s", [128, KT * 340], BF)
        bqk_s = sb("bqk_s", [128, 8], F32)
        bv_s = sb("bv_s", [1, 340], BF)
        cs_cc_s = sb("cs_cc_s", [128, T], BF)
        cs_ss_s = sb("cs_ss_s", [128, T], BF)
        bo_s = sb("bo_s", [128, D], F32)
        if mode == "causal":
            dmask_s = sb("dmask_s", [128, 4 * CW], BF)
        if mode == "general":
            fm_s = sb("fm_s", [128, KT * CW], BF)
        xs_ = sb("xs_", [128, 2 * KT * CW], BF)   # x chunk double buffer (aliased by wo bufs in phase 4)
        qT_s = sb("qT_s", [128, HL * T], BF)
        kT_s = sb("kT_s", [128, HL * T], BF)
        vg_s = sb("vg_s", [128, NTT * HL * DVP], BF)
        shf_s = sb("shf_s", [128, CW], F32)
        A_s = sb("A_s", [128, CW], F32)
        B_s = sb("B_s", [128, CW], F32)
        pT = [sb(f"pT{i}", [128, CW], BF) for i in range(4)]
        oT_stage = [sb(f"oT_stage{i}", [DV, CW], BF) for i in range(2)]
        oT_f32 = [sb(f"oT_f32_{i}", [DVP, CW], F32) for i in range(2)]
        recip_fr = [sb(f"recip_fr{i}", [1, CW], BF) for i in range(2)]
        den_s = [sb(f"den_s{i}", [1, CW], F32) for i in range(2)]
        ones_bf = sb("ones_bf", [1, 128], BF)
        ao_s = sb("ao_s", [128, NKT_WO * CW], BF)
        out_s = [sb(f"out_s{i}", [128, CW], F32) for i in range(2)]
        # wo double buffer aliases the (dead in phase 4) x-chunk buffer
        wo_buf = xs_

        ps = [es.enter_context(nc.psum_tensor(f"ps{i}", [128, CW], F32)) for i in range(8)]
        # phase1: q/k banks 0-3 keyed (chunk%2, h); v banks 4-7 keyed t%4
        # phase2: sT banks 0/1, oT banks 2/3, bcast banks 4/5
        # phase4: out banks 6/7

        sem_names = [
            "s_pre", "s_x", "s_qk", "s_rope", "s_v", "s_silu", "s_glu",
            "s_sT", "s_exp", "s_mask", "s_pv", "s_ocp", "s_den", "s_recip", "s_bcast",
            "s_norm", "s_oT", "s_cc", "s_ao", "s_wo_pe", "s_evac",
            "s_out", "s_misc",
        ]
        if mode == "general":
            sem_names.append("s_fm")
        sm = {n: es.enter_context(nc.semaphore(n)) for n in sem_names}
        block = es.enter_context(nc.Block())

        x_stream = [("xq_t", c) for c in range(NCHUNK)] + \
                   [("xk_t", c) for c in range(NCHUNK)] + \
                   [("xv_t", c) for c in range(NCHUNK)]

        # ================= SYNC: DMA engine =================
        @block.sync
        def _(sync):
            for name in pre_order:
                dst = {"wq": wq_s, "wk": wk_s, "wv": wv_s, "bqk": bqk_s,
                       "bv": bv_s, "cs_cc": cs_cc_s, "cs_ss": cs_ss_s,
                       "bo_rep": bo_s}.get(name)
                if name == "dmask":
                    dst = dmask_s
                src = d[name]
                if name in ("wq", "wk", "wv"):
                    w = 256 if name != "wv" else 340
                    sync.dma_start(
                        out=dst[:].rearrange("p (kt w) -> p kt w", kt=KT),
                        in_=src.rearrange("(kt p) w -> p kt w", p=128),
                    ).then_inc(sm["s_pre"], 16)
                else:
                    sync.dma_start(out=dst[:], in_=src[:]).then_inc(sm["s_pre"], 16)

            # x chunk stream
            for gc, (name, c) in enumerate(x_stream):
                buf = gc % 2
                # credit: wait until the previous occupant (chunk gc-2) consumed
                pc = gc - 2
                if pc >= 0:
                    if pc < 16:
                        sync.wait_ge(sm["s_qk"], 2 * pc + 2)
                    else:
                        sync.wait_ge(sm["s_v"], 4 * (pc - 16) + 4)
                sync.dma_start(
                    out=xs_[:, buf * KT * CW:(buf + 1) * KT * CW]
                        .rearrange("p (kt w) -> p kt w", kt=KT),
                    in_=d[name].rearrange("(kt p) t -> p kt t", p=128)
                        [:, :, c * CW:(c + 1) * CW],
                ).then_inc(sm["s_x"], 16)

            # general mode: fmask chunks loaded during phase 2 (qc-outer order)
            if mode == "general":
                for qc in range(4):
                    if qc >= 1:
                        sync.wait_ge(sm["s_mask"], 64 * qc)
                    sync.dma_start(
                        out=fm_s[:].rearrange("p (kt w) -> p kt w", kt=KT),
                        in_=d["fmask"].rearrange("(kt p) t -> p kt t", p=128)
                            [:, :, qc * CW:(qc + 1) * CW],
                    ).then_inc(sm["s_fm"], 16)

            # oT stores to a2a input buffers (+ denominator row bounce
            # to partition 0 -- per-lane engines cannot cross partitions)
            for i2, (h, b, qc) in enumerate(iters):
                t = b * 4 + qc
                sync.wait_ge(sm["s_ocp"], i2 + 1)
                if i2 - 1 >= 1:
                    sync.wait_ge(sm["s_recip"], i2 - 1)
                sync.dma_start(out=den_s[i2 % 2][:], in_=oT_f32[i2 % 2][DEN:DEN + 1, :]
                               ).then_inc(sm["s_den"], 16)
                sync.wait_ge(sm["s_norm"], i2 + 1)
                sync.dma_start(out=a2a_in[h][t, :, :], in_=oT_stage[i2 % 2][:]
                               ).then_inc(sm["s_oT"], 16)

            # phase 4 loads: wo n=0,1 can start now (buffer aliased on xs_)
            for n in (0, 1):
                sync.dma_start(
                    out=wo_buf[:, n * NKT_WO * CW:(n + 1) * NKT_WO * CW]
                        .rearrange("p (kt w) -> p kt w", kt=NKT_WO),
                    in_=d["wo"][:, :, n * CW:(n + 1) * CW]
                        .rearrange("kt p w -> p kt w"),
                ).then_inc(sm["s_ao"], 16)
            # ao loads after collectives
            for g in range(2):
                sync.wait_ge(sm["s_cc"], g + 1)
                flat = a2a_out[g].rearrange("r p t -> (r p) t")
                sync.dma_start(
                    out=ao_s[:, (g * 6) * CW:(g * 6 + 5) * CW]
                        .rearrange("p (kt w) -> p kt w", kt=5),
                    in_=flat[0:640].rearrange("(kt p) t -> p kt t", p=128),
                ).then_inc(sm["s_ao"], 16)
                sync.dma_start(
                    out=ao_s[0:40, (g * 6 + 5) * CW:(g * 6 + 6) * CW],
                    in_=flat[640:680],
                ).then_inc(sm["s_ao"], 16)
            # wo n=2 (after PE consumed n=0), out stores 0-3, wo n=3, rest
            sync.wait_ge(sm["s_wo_pe"], 4)
            sync.dma_start(
                out=wo_buf[:, 0:NKT_WO * CW]
                    .rearrange("p (kt w) -> p kt w", kt=NKT_WO),
                in_=d["wo"][:, :, 2 * CW:3 * CW].rearrange("kt p w -> p kt w"),
            ).then_inc(sm["s_ao"], 16)
            for idx in range(4):
                n, m = idx // 4, idx % 4
                sync.wait_ge(sm["s_evac"], idx + 1)
                sync.dma_start(out=out_d[m * 128:(m + 1) * 128, n * CW:(n + 1) * CW],
                               in_=out_s[idx % 2][:]).then_inc(sm["s_out"], 16)
            sync.wait_ge(sm["s_wo_pe"], 8)
            sync.dma_start(
                out=wo_buf[:, NKT_WO * CW:2 * NKT_WO * CW]
                    .rearrange("p (kt w) -> p kt w", kt=NKT_WO),
                in_=d["wo"][:, :, 3 * CW:4 * CW].rearrange("kt p w -> p kt w"),
            ).then_inc(sm["s_ao"], 16)
            for idx in range(4, 16):
                n, m = idx // 4, idx % 4
                sync.wait_ge(sm["s_evac"], idx + 1)
                sync.dma_start(out=out_d[m * 128:(m + 1) * 128, n * CW:(n + 1) * CW],
                               in_=out_s[idx % 2][:]).then_inc(sm["s_out"], 16)
            sync.wait_ge(sm["s_out"], 16 * 16)
            if debug:
                sync.dma_start(out=dbg["dbg_qT"][:], in_=qT_s[:]).then_inc(sm["s_out"], 16)
                sync.dma_start(out=dbg["dbg_kT"][:], in_=kT_s[:]).then_inc(sm["s_out"], 16)
                sync.dma_start(out=dbg["dbg_vg"][:], in_=vg_s[:]).then_inc(sm["s_out"], 16)
                sync.dma_start(out=dbg["dbg_ao"][:], in_=ao_s[:]).then_inc(sm["s_out"], 16)
                sync.dma_start(out=dbg["dbg_oT"][:], in_=oT_f32[(NIT - 1) % 2][:]).then_inc(sm["s_out"], 16)
                li = (NIT - 1) % 2
                sync.dma_start(out=dbg["dbg_nrm"][:], in_=oT_stage[li][:]).then_inc(sm["s_out"], 16)
                sync.dma_start(out=dbg["dbg_rcp"][:], in_=recip_fr[li][:]).then_inc(sm["s_out"], 16)
                sync.wait_ge(sm["s_out"], 16 * 23)

        # ================= TENSOR: PE =================
        @block.tensor
        def _(tensor):
            mm = nc.tensor.matmul

            def bcast_emit(i):
                tensor.wait_ge(sm["s_recip"], i + 1)
                if i >= 1:
                    tensor.wait_ge(sm["s_norm"], i)
                mm(ps[4][0:DV, :], ones_bf[0:1, 0:DV],
                   recip_fr[i % 2][:], start=True, stop=True
                   ).then_inc(sm["s_bcast"], 1)

            # ---- phase 1: q/k transposed projections ----
            for tens_i, (xname, wsb, pre_need) in enumerate(
                    [("q", wq_s, PRE["wq"]), ("k", wk_s, PRE["wk"])]):
                for c in range(NCHUNK):
                    gc = tens_i * NCHUNK + c
                    for h in range(HL):
                        gidx = gc * HL + h
                        if gidx == 0:
                            tensor.wait_ge(sm["s_pre"], pre_need)
                        elif h == 0 and c == 0:
                            tensor.wait_ge(sm["s_pre"], pre_need)
                        if h == 0:
                            tensor.wait_ge(sm["s_x"], 16 * (gc + 1))
                        if gidx - 3 >= 1:
                            tensor.wait_ge(sm["s_rope"], gidx - 3)
                        bank = ps[(gc % 2) * 2 + h]
                        buf = gc % 2
                        for kt in range(KT):
                            mm(bank[:],
                               wsb[:, kt * 256 + h * 128: kt * 256 + (h + 1) * 128],
                               xs_[:, buf * KT * CW + kt * CW: buf * KT * CW + (kt + 1) * CW],
                               start=(kt == 0), stop=(kt == KT - 1)
                               ).then_maybe_inc((sm["s_qk"], 1) if kt == KT - 1 else None)

            # ---- phase 1: v projection + bias ----
            tensor.wait_ge(sm["s_pre"], PRE["bv"])
            tensor.wait_ge(sm["s_misc"], 1)
            for t in range(NTT):
                gc = 16 + t // 4
                if t % 4 == 0:
                    tensor.wait_ge(sm["s_x"], 16 * (gc + 1))
                if t - 3 >= 1:
                    tensor.wait_ge(sm["s_glu"], t - 3)
                bank = ps[4 + t % 4]
                buf = gc % 2
                toff = buf * KT * CW + (t % 4) * 128
                for kt in range(KT):
                    mm(bank[:, 0:340],
                       xs_[:, toff + kt * CW: toff + kt * CW + 128],
                       wv_s[:, kt * 340:(kt + 1) * 340],
                       start=(kt == 0), stop=False)
                mm(bank[:, 0:340], ones_bf[0:1, 0:128], bv_s[:],
                   start=False, stop=True).then_inc(sm["s_v"], 1)

            # ---- phase 2: attention (software-pipelined, PV lags sT by
            # LAG tiles so the exp/mask waits are pre-satisfied) ----
            tensor.wait_ge(sm["s_rope"], 32)
            tensor.wait_ge(sm["s_glu"], 32)
            LAG = 2
            sT_banks = [ps[0], ps[1], ps[5]]

            def emit_sT(gj):
                i2, jj, nk, msk = tiles[gj]
                h, b, qc = iters[i2]
                mm(sT_banks[gj % 3][:],
                   kT_s[:, h * T + b * S + jj * 128: h * T + b * S + (jj + 1) * 128],
                   qT_s[:, h * T + b * S + qc * CW: h * T + b * S + qc * CW + CW],
                   start=True, stop=True).then_inc(sm["s_sT"], 1)

            def emit_pv(gj):
                i2, jj, nk, msk = tiles[gj]
                h, b, qc = iters[i2]
                if msk:
                    tensor.wait_ge(sm["s_mask"], cum_mask[gj])
                else:
                    tensor.wait_ge(sm["s_exp"], gj + 1)
                if jj == 0 and i2 - 1 >= 1:
                    tensor.wait_ge(sm["s_ocp"], i2 - 1)
                g = b * 16 + jj
                mm(ps[2 + i2 % 2][0:DVP, :],
                   vg_s[:, (g * HL + h) * DVP: (g * HL + h) * DVP + DVP],
                   pT[gj % 4][:],
                   start=(jj == 0), stop=(jj == nk - 1)
                   ).then_inc(sm["s_pv"], 1)

            from collections import deque
            pvq = deque()
            bcast_due = {}       # op_idx -> iter index
            opi = 0

            def flush_due():
                nonlocal opi
                for at in sorted(list(bcast_due)):
                    if at <= opi:
                        bcast_emit(bcast_due.pop(at))

            for gj in range(NTILE):
                flush_due()
                emit_sT(gj); opi += 1
                pvq.append(gj)
                if len(pvq) > LAG:
                    flush_due()
                    g2 = pvq.popleft()
                    emit_pv(g2); opi += 1
                    if g2 == iter_last[tiles[g2][0]]:
                        bcast_due[opi + 9] = tiles[g2][0]
            while pvq:
                flush_due()
                g2 = pvq.popleft()
                emit_pv(g2); opi += 1
                if g2 == iter_last[tiles[g2][0]]:
                    bcast_due[opi + 9] = tiles[g2][0]
            for at in sorted(list(bcast_due)):
                bcast_emit(bcast_due.pop(at))

            # ---- phase 4: Wo ----
            for idx in range(16):
                n, m = idx // 4, idx % 4
                need = {0: 96, 1: 96, 2: 112, 3: 128}[n]
                if m == 0:
                    tensor.wait_ge(sm["s_ao"], need)
                if idx - 1 >= 1:
                    tensor.wait_ge(sm["s_evac"], idx - 1)
                nb = n % 2
                for kt in range(NKT_WO):
                    mm(ps[6 + idx % 2][:],
                       ao_s[:, kt * CW + m * 128: kt * CW + (m + 1) * 128],
                       wo_buf[:, nb * NKT_WO * CW + kt * CW: nb * NKT_WO * CW + (kt + 1) * CW],
                       start=(kt == 0), stop=(kt == NKT_WO - 1)
                       ).then_maybe_inc((sm["s_wo_pe"], 1) if kt == NKT_WO - 1 else None)

        # ================= SCALAR: ACT =================
        @block.scalar
        def _(scalar):
            act = nc.scalar.activation

            def ocp_emit(i):
                # evacuate oT'+denom PSUM -> SBUF f32 (frees oT bank, enables
                # single-PSUM-operand DVE ops downstream)
                scalar.wait_ge(sm["s_pv"], iter_last[i] + 1)
                if i - 1 >= 1:
                    scalar.wait_ge(sm["s_norm"], i - 1)
                nc.scalar.copy(oT_f32[i % 2][:], ps[2 + i % 2][0:DVP, :]
                               ).then_inc(sm["s_ocp"], 1)

            # phase 1: silu for GLU
            for t in range(NTT):
                scalar.wait_ge(sm["s_v"], t + 1)
                if t - 1 >= 1:
                    scalar.wait_ge(sm["s_glu"], t - 1)
                act(pT[t % 2][:, 0:170], ps[4 + t % 4][:, 170:340], Silu
                    ).then_inc(sm["s_silu"], 1)
            # phase 2: exp (+ oT evacuation copies; ocp(i) goes after the
            # 2nd exp of iter i+1 so its s_pv wait is pre-satisfied under
            # the PE's lagged PV schedule)
            sT_banks_a = [ps[0], ps[1], ps[5]]
            ocp_after = {}
            for i in range(1, NIT):
                ocp_after[min(iter_first[i] + 1, iter_last[i])] = i - 1
            for gj in range(NTILE):
                scalar.wait_ge(sm["s_sT"], gj + 1)
                if gj - 3 >= 1:
                    scalar.wait_ge(sm["s_pv"], gj - 3)
                act(pT[gj % 4][:], sT_banks_a[gj % 3][:], Exp, scale=SCALE
                    ).then_inc(sm["s_exp"], 1)
                if gj in ocp_after:
                    ocp_emit(ocp_after[gj])
            ocp_emit(NIT - 1)

        # ================= VECTOR: DVE =================
        @block.vector
        def _(vector):
            vec = nc.vector
            # init constants
            vec.memset(vg_s[:], 1.0)
            vec.memset(ones_bf[:], 1.0)
            vec.memset(ao_s[:, 5 * CW:6 * CW], 0.0)
            vec.memset(ao_s[:, 11 * CW:12 * CW], 0.0).then_inc(sm["s_misc"], 1)

            # phase 1: rope for q then k
            vector.wait_ge(sm["s_pre"], PRE["cs_ss"])
            for gidx in range(32):
                tens_i, rem = divmod(gidx, 16)
                c, h = divmod(rem, HL)
                gc = tens_i * NCHUNK + c
                bank = ps[(gc % 2) * 2 + h]
                dst = (qT_s if tens_i == 0 else kT_s)
                toff = c * CW
                bcol = tens_i * 4 + h
                vector.wait_ge(sm["s_qk"], gidx + 1)
                vec.stream_shuffle(shf_s[:], bank[:], SWAP_MASK)
                vec.scalar_tensor_tensor(
                    A_s[:], bank[:], bqk_s[:, bcol:bcol + 1],
                    cs_cc_s[:, toff:toff + CW], op0=ADD, op1=MULT)
                vec.scalar_tensor_tensor(
                    B_s[:], shf_s[:], bqk_s[:, bcol + 2:bcol + 3],
                    cs_ss_s[:, toff:toff + CW], op0=ADD, op1=MULT)
                vec.tensor_add(dst[:, h * T + toff: h * T + toff + CW],
                               A_s[:], B_s[:]).then_inc(sm["s_rope"], 1)

            # phase 1: GLU muls
            for t in range(NTT):
                vector.wait_ge(sm["s_silu"], t + 1)
                for h in range(HL):
                    ins = vec.tensor_mul(
                        vg_s[:, (t * HL + h) * DVP:(t * HL + h) * DVP + DV],
                        ps[4 + t % 4][:, h * DV:(h + 1) * DV],
                        pT[t % 2][:, h * DV:(h + 1) * DV])
                    if h == HL - 1:
                        ins.then_inc(sm["s_glu"], 1)

            # phase 2: masks / recip / norm
            def norm_emit(i):
                vector.wait_ge(sm["s_bcast"], i + 1)
                if 16 * (i - 1) >= 16:
                    vector.wait_ge(sm["s_oT"], 16 * (i - 1))
                vec.tensor_mul(oT_stage[i % 2][:], oT_f32[i % 2][0:DV, :],
                               ps[4][0:DV, :]).then_inc(sm["s_norm"], 1)

            for i2, (h, b, qc) in enumerate(iters):
                nk = nk_of(qc)
                for jj in range(nk):
                    gj = iter_first[i2] + jj
                    if tiles[gj][3]:
                        vector.wait_ge(sm["s_exp"], gj + 1)
                        if mode == "causal":
                            r = jj - 4 * qc
                            msrc = dmask_s[:, r * CW:(r + 1) * CW]
                        else:
                            vector.wait_ge(sm["s_fm"], 16 * (qc + 1))
                            msrc = fm_s[:, jj * CW:(jj + 1) * CW]
                        vec.tensor_mul(pT[gj % 4][:], pT[gj % 4][:], msrc
                                       ).then_inc(sm["s_mask"], 1)
                # norm for the previous iteration must precede this recip:
                # recip depends on sync's den-bounce DMA, and sync only
                # reaches it after storing iter i2-1 (which needs norm(i2-1)).
                if i2 >= 1:
                    norm_emit(i2 - 1)
                # recip for this iteration (reads the DMA-bounced denom row)
                vector.wait_ge(sm["s_den"], 16 * (i2 + 1))
                if i2 - 1 >= 1:
                    vector.wait_ge(sm["s_bcast"], i2 - 1)
                with nc.allow_low_precision(reason="softmax denom recip in bf16"):
                    vec.reciprocal(recip_fr[i2 % 2][:], den_s[i2 % 2][:]
                                   ).then_inc(sm["s_recip"], 1)
            norm_emit(NIT - 1)

            # phase 4: evac + bias
            vector.wait_ge(sm["s_pre"], PRE["bo_rep"])
            for idx in range(16):
                n = idx // 4
                vector.wait_ge(sm["s_wo_pe"], idx + 1)
                if 16 * (idx - 1) >= 16:
                    vector.wait_ge(sm["s_out"], 16 * (idx - 1))
                vec.tensor_add(out_s[idx % 2][:], ps[6 + idx % 2][:],
                               bo_s[:, n * CW:(n + 1) * CW]).then_inc(sm["s_evac"], 1)

        # ================= GPSIMD: collectives =================
        @block.gpsimd
        def _(gpsimd):
            rg = [list(range(NCORE))]
            for g in range(2):
                gpsimd.wait_ge(sm["s_oT"], 16 * 8 * (g + 1) if mode != "general" else 16 * 16)
                gpsimd.collective_compute(
                    "AllToAll", mybir.AluOpType.bypass,
                    replica_groups=rg,
                    ins=[a2a_in[g][:]], outs=[a2a_out[g][:]],
                ).then_inc(sm["s_cc"], 1)

    return nc


_cache = {}


def _get_nc(mode):
    if mode not in _cache:
        _cache[mode] = build(mode)
    return _cache[mode]


def kernel(q, k, v, mask, Wq, bq, Wk, bk, Wv, bv, Wo, bo):
    from concourse.bass_utils import run_bass_kernel_spmd

    q = np.asarray(q, np.float32)
    k = np.asarray(k, np.float32)
    v = np.asarray(v, np.float32)
    mask = np.asarray(mask)
    Wq = np.asarray(Wq, np.float32); bq = np.asarray(bq, np.float32)
    Wk = np.asarray(Wk, np.float32); bk = np.asarray(bk, np.float32)
    Wv = np.asarray(Wv, np.float32); bv = np.asarray(bv, np.float32)
    Wo = np.asarray(Wo, np.float32); bo = np.asarray(bo, np.float32)

    causal_ref = np.triu(np.ones((S, S), bool), 1)
    if mask.shape == (S, S) and (mask == causal_ref).all():
        mode = "causal"
    elif not mask.any():
        mode = "nomask"
    else:
        mode = "general"

    in_maps, dmask = _host_inputs(q, k, v, mask, Wq, bq, Wk, bk, Wv, bv, Wo, bo)
    if mode == "causal":
        for m in in_maps:
            m["dmask"] = dmask
    elif mode == "general":
        fm = np.where(mask, 0.0, 1.0).T.astype(bf16)   # [k, q] multiplicative
        fm = np.ascontiguousarray(fm)
        for m in in_maps:
            m["fmask"] = fm

    nc = _get_nc(mode)
    tmpdir = os.environ.get("BASS_TMPDIR")
    if tmpdir:
        os.makedirs(tmpdir, exist_ok=True)
    res = run_bass_kernel_spmd(nc, in_maps, list(range(NCORE)), tmpdir=tmpdir)
    LAST["exec_time_ns"] = res.exec_time_ns
    LAST["results"] = res
    out = np.concatenate([res.results[c]["out"] for c in range(NCORE)], axis=0)
    return np.ascontiguousarray(out.reshape(B, S, D).astype(np.float32))


# revision 30
# speedup vs baseline: 1.5216x; 1.0045x over previous
"""Trainium2 Bass kernel for nn_MultiHeadAttention_4690104287263.

Strategy (8 NeuronCores, SPMD):
  - Tensor-parallel over heads: each core owns 2 of the 16 heads.
  - Projections computed TRANSPOSED (qT/kT: [dqk, tok]) so the attention
    matmuls need no on-device transposes. RoPE pairs are interleaved into
    adjacent partitions (baked into the host-side weight column order) so
    the half-rotation becomes a DVE stream_shuffle (even/odd lane swap).
  - Attention computed transposed: sT[k, q] = kT.T @ qT, softmax over the
    partition (k) dim using exp (no max subtraction; scores are O(1)) and
    a ones-column appended to V so the PV matmul also yields the softmax
    denominator for free. Block-causal: upper k-tiles skipped, diagonal
    tiles masked multiplicatively after exp.
  - AllToAll (2 calls, one per local head) reshards attention output from
    head-sharded to token-sharded; Wo is then computed token-sharded with
    the full (zero-padded, even/odd-head-grouped) Wo. No all-reduce.
  - bf16 storage/matmuls with f32 PSUM accumulation throughout.
"""

import os
import numpy as np
import ml_dtypes

bf16 = ml_dtypes.bfloat16

B, S, D, H, DQK, DV = 2, 2048, 2048, 16, 128, 85
T = B * S                 # 4096 flat tokens
NCORE = 8
HL = 2                    # heads per core
SCALE = float(DQK) ** -0.5
ROPE_BASE = 10000.0
CW = 512                  # token chunk width
NCHUNK = T // CW          # 8
KT = D // 128             # 16 k-tiles over d_model
NTT = T // 128            # 32 token tiles
DVP = 97                  # dv(85) + pad + ones column at DEN (32-aligned)
DEN = 96                  # denominator row (must be a legal partition start)
NKT_WO = 12               # Wo K-tiles (2 groups x 6, zero padded 680->768)

SWAP_MASK = [i ^ 1 for i in range(32)]

LAST = {"exec_time_ns": None, "results": None}


def _host_inputs(q, k, v, mask, Wq, bq, Wk, bk, Wv, bv, Wo, bo):
    """Prepare per-core input maps (host-side shard/layout/cast)."""
    f32 = np.float32
    # x^T layouts [D, T], bf16
    xq_t = np.ascontiguousarray(q.reshape(T, D).T).astype(bf16)
    xk_t = np.ascontiguousarray(k.reshape(T, D).T).astype(bf16)
    xv_t = np.ascontiguousarray(v.reshape(T, D).T).astype(bf16)

    # rope pair-interleave permutation within each head's 128 cols
    perm = np.empty(128, np.int64)
    perm[0::2] = np.arange(64)
    perm[1::2] = np.arange(64) + 64

    # cos/sin tables in interleaved layout [128, T]
    inv_freq = 1.0 / (ROPE_BASE ** (np.arange(64, dtype=np.float64) / 64.0))
    pos = np.arange(S, dtype=np.float64)
    ang = pos[:, None] * inv_freq[None, :]           # [S, 64]
    cos = np.cos(ang).T                              # [64, S]
    sin = np.sin(ang).T
    cs_cc = np.empty((128, T), f32)
    cs_ss = np.empty((128, T), f32)
    for b in range(B):
        sl = slice(b * S, (b + 1) * S)
        cs_cc[0::2, sl] = cos
        cs_cc[1::2, sl] = cos
        cs_ss[0::2, sl] = -sin
        cs_ss[1::2, sl] = sin
    cs_cc = cs_cc.astype(bf16)
    cs_ss = cs_ss.astype(bf16)

    # diagonal-block causal multiplicative masks [128, 4*512]
    p_i = np.arange(128)[:, None]
    c_i = np.arange(CW)[None, :]
    dmask = np.concatenate(
        [(p_i <= c_i - 128 * r).astype(f32) for r in range(4)], axis=1
    ).astype(bf16)

    # Wo grouped even/odd heads, K-padded to 12 x 128 rows
    Wo3 = Wo.reshape(H, DV, D)
    wo_g = []
    for par in (0, 1):
        g = Wo3[par::2].reshape(8 * DV, D)           # [680, D]
        gp = np.zeros((6 * 128, D), f32)
        gp[:680] = g
        wo_g.append(gp)
    wo_tiled = np.concatenate(wo_g, axis=0).reshape(NKT_WO, 128, D).astype(bf16)

    bo_rep = np.broadcast_to(bo.astype(f32), (128, D)).copy()

    in_maps = []
    for c in range(NCORE):
        heads = [2 * c, 2 * c + 1]
        wq_c = np.empty((D, 256), f32)
        wk_c = np.empty((D, 256), f32)
        bqk_c = np.empty((128, 8), f32)
        for hl, hg in enumerate(heads):
            wq_c[:, hl * 128:(hl + 1) * 128] = Wq[:, hg * 128:(hg + 1) * 128][:, perm]
            wk_c[:, hl * 128:(hl + 1) * 128] = Wk[:, hg * 128:(hg + 1) * 128][:, perm]
            bq_p = bq[hg * 128:(hg + 1) * 128][perm]
            bk_p = bk[hg * 128:(hg + 1) * 128][perm]
            sw = np.arange(128) ^ 1
            bqk_c[:, hl] = bq_p
            bqk_c[:, 2 + hl] = bq_p[sw]
            bqk_c[:, 4 + hl] = bk_p
            bqk_c[:, 6 + hl] = bk_p[sw]
        lin = slice(2 * c * DV, 2 * c * DV + 2 * DV)
        gate = slice(H * DV + 2 * c * DV, H * DV + 2 * c * DV + 2 * DV)
        wv_c = np.concatenate([Wv[:, lin], Wv[:, gate]], axis=1)   # [D, 340]
        bv_c = np.concatenate([bv[lin], bv[gate]])[None, :]        # [1, 340]
        m = {
            "xq_t": xq_t, "xk_t": xk_t, "xv_t": xv_t,
            "wq": wq_c.astype(bf16), "wk": wk_c.astype(bf16),
            "wv": wv_c.astype(bf16),
            "bqk": bqk_c, "bv": bv_c.astype(bf16),
            "cs_cc": cs_cc, "cs_ss": cs_ss,
            "wo": wo_tiled, "bo_rep": bo_rep,
        }
        in_maps.append(m)
    return in_maps, dmask


def build(mode, debug=False):
    """mode in {'causal', 'nomask', 'general'}"""
    import concourse.bass as bass
    import concourse.mybir as mybir
    from contextlib import ExitStack

    F32 = mybir.dt.float32
    F32R = mybir.dt.float32r
    BF = mybir.dt.bfloat16
    Exp = mybir.ActivationFunctionType.Exp
    Silu = mybir.ActivationFunctionType.Silu
    ADD = mybir.AluOpType.add
    MULT = mybir.AluOpType.mult

    nc = bass.Bass()
    d = {}
    d["xq_t"] = nc.dram_tensor("xq_t", [D, T], BF, kind="ExternalInput")
    d["xk_t"] = nc.dram_tensor("xk_t", [D, T], BF, kind="ExternalInput")
    d["xv_t"] = nc.dram_tensor("xv_t", [D, T], BF, kind="ExternalInput")
    d["wq"] = nc.dram_tensor("wq", [D, 256], BF, kind="ExternalInput")
    d["wk"] = nc.dram_tensor("wk", [D, 256], BF, kind="ExternalInput")
    d["wv"] = nc.dram_tensor("wv", [D, 340], BF, kind="ExternalInput")
    d["bqk"] = nc.dram_tensor("bqk", [128, 8], F32, kind="ExternalInput")
    d["bv"] = nc.dram_tensor("bv", [1, 340], BF, kind="ExternalInput")
    d["cs_cc"] = nc.dram_tensor("cs_cc", [128, T], BF, kind="ExternalInput")
    d["cs_ss"] = nc.dram_tensor("cs_ss", [128, T], BF, kind="ExternalInput")
    d["wo"] = nc.dram_tensor("wo", [NKT_WO, 128, D], BF, kind="ExternalInput")
    d["bo_rep"] = nc.dram_tensor("bo_rep", [128, D], F32, kind="ExternalInput")
    if mode == "causal":
        d["dmask"] = nc.dram_tensor("dmask", [128, 4 * CW], BF, kind="ExternalInput")
    if mode == "general":
        d["fmask"] = nc.dram_tensor("fmask", [S, S], BF, kind="ExternalInput")
    out_d = nc.dram_tensor("out", [CW, D], F32, kind="ExternalOutput")
    if debug:
        dbg = {
            "dbg_qT": nc.dram_tensor("dbg_qT", [128, HL * T], BF, kind="ExternalOutput"),
            "dbg_kT": nc.dram_tensor("dbg_kT", [128, HL * T], BF, kind="ExternalOutput"),
            "dbg_vg": nc.dram_tensor("dbg_vg", [128, NTT * HL * DVP], BF, kind="ExternalOutput"),
            "dbg_ao": nc.dram_tensor("dbg_ao", [128, NKT_WO * CW], BF, kind="ExternalOutput"),
            "dbg_oT": nc.dram_tensor("dbg_oT", [DVP, CW], F32, kind="ExternalOutput"),
            "dbg_nrm": nc.dram_tensor("dbg_nrm", [DV, CW], BF, kind="ExternalOutput"),
            "dbg_rcp": nc.dram_tensor("dbg_rcp", [1, CW], BF, kind="ExternalOutput"),
        }
    a2a_in = [nc.dram_tensor(f"a2a_in{h}", [NCORE, DV, CW], BF) for h in range(HL)]
    a2a_out = [nc.dram_tensor(f"a2a_out{h}", [NCORE, DV, CW], BF) for h in range(HL)]

    # ---- iteration schedules -------------------------------------------
    if mode == "general":
        iters = [(h, b, qc) for qc in range(4) for b in range(B) for h in range(HL)]
    else:
        iters = [(h, b, qc) for h in range(HL) for b in range(B) for qc in range(4)]

    def nk_of(qc):
        return 4 * qc + 4 if mode == "causal" else 16

    def is_masked(qc, j):
        if mode == "causal":
            return j >= 4 * qc
        return mode == "general"

    # per-tile bookkeeping (global tile index -> cumulative mask count)
    tiles = []           # list of (i2, jj, nk, masked)
    cum_mask = []
    cm = 0
    for i2, (h, b, qc) in enumerate(iters):
        nk = nk_of(qc)
        for jj in range(nk):
            msk = is_masked(qc, jj)
            if msk:
                cm += 1
            tiles.append((i2, jj, nk, msk))
            cum_mask.append(cm)
    NTILE = len(tiles)
    # first global tile index of each iteration, and last
    iter_first = {}
    iter_last = {}
    for gj, (i2, jj, nk, msk) in enumerate(tiles):
        if jj == 0:
            iter_first[i2] = gj
        if jj == nk - 1:
            iter_last[i2] = gj
    NIT = len(iters)

    # preload order -> s_pre thresholds
    pre_order = ["wq", "wk", "wv", "bqk", "bv", "cs_cc", "cs_ss", "bo_rep"]
    if mode == "causal":
        pre_order.append("dmask")
    PRE = {name: 16 * (i + 1) for i, name in enumerate(pre_order)}

    with ExitStack() as es:
        def sb(name, shape, dt_):
            return es.enter_context(nc.sbuf_tensor(name, shape, dt_))
        wq_s = sb("wq_s", [128, KT * 256], BF)
        wk_s = sb("wk_s", [128, KT * 256], BF)
        wv_s = sb("wv_s", [128, KT * 340], BF)
        bqk_s = sb("bqk_s", [128, 8], F32)
        bv_s = sb("bv_s", [1, 340], BF)
        cs_cc_s = sb("cs_cc_s", [128, T], BF)
        cs_ss_s = sb("cs_ss_s", [128, T], BF)
        bo_s = sb("bo_s", [128, D], F32)
        if mode == "causal":
            dmask_s = sb("dmask_s", [128, 4 * CW], BF)
        if mode == "general":
            fm_s = sb("fm_s", [128, KT * CW], BF)
        xs_ = sb("xs_", [128, 2 * KT * CW], BF)   # x chunk double buffer (aliased by wo bufs in phase 4)
        qT_s = sb("qT_s", [128, HL * T], BF)
        kT_s = sb("kT_s", [128, HL * T], BF)
        vg_s = sb("vg_s", [128, NTT * HL * DVP], BF)
        shf_s = sb("shf_s", [128, CW], F32)
        A_s = sb("A_s", [128, CW], F32)
        B_s = sb("B_s", [128, CW], F32)
        pT = [sb(f"pT{i}", [128, CW], BF) for i in range(6)]
        oT_stage = [sb(f"oT_stage{i}", [DV, CW], BF) for i in range(2)]
        oT_f32 = [sb(f"oT_f32_{i}", [DVP, CW], F32) for i in range(2)]
        recip_fr = [sb(f"recip_fr{i}", [1, CW], BF) for i in range(2)]
        den_s = [sb(f"den_s{i}", [1, CW], F32) for i in range(2)]
        ones_bf = sb("ones_bf", [1, 128], BF)
        ao_s = sb("ao_s", [128, NKT_WO * CW], BF)
        out_s = [sb(f"out_s{i}", [128, CW], F32) for i in range(2)]
        # wo double buffer aliases the (dead in phase 4) x-chunk buffer
        wo_buf = xs_

        ps = [es.enter_context(nc.psum_tensor(f"ps{i}", [128, CW], F32)) for i in range(8)]
        # phase1: q/k banks 0-3 keyed (chunk%2, h); v banks 4-7 keyed t%4
        # phase2: sT banks 0/1, oT banks 2/3, bcast banks 4/5
        # phase4: out banks 6/7

        sem_names = [
            "s_pre", "s_x", "s_qk", "s_rope", "s_v", "s_silu", "s_glu",
            "s_sT", "s_exp", "s_mask", "s_pv", "s_ocp", "s_den", "s_recip", "s_bcast",
            "s_norm", "s_oT", "s_cc", "s_ao", "s_wo_pe", "s_evac",
            "s_out", "s_misc",
        ]
        if mode == "general":
            sem_names.append("s_fm")
        sm = {n: es.enter_context(nc.semaphore(n)) for n in sem_names}
        block = es.enter_context(nc.Block())

        x_stream = [("xq_t", c) for c in range(NCHUNK)] + \
                   [("xk_t", c) for c in range(NCHUNK)] + \
                   [("xv_t", c) for c in range(NCHUNK)]

        # ================= SYNC: DMA engine =================
        @block.sync
        def _(sync):
            for name in pre_order:
                dst = {"wq": wq_s, "wk": wk_s, "wv": wv_s, "bqk": bqk_s,
                       "bv": bv_s, "cs_cc": cs_cc_s, "cs_ss": cs_ss_s,
                       "bo_rep": bo_s}.get(name)
                if name == "dmask":
                    dst = dmask_s
                src = d[name]
                if name in ("wq", "wk", "wv"):
                    w = 256 if name != "wv" else 340
                    sync.dma_start(
                        out=dst[:].rearrange("p (kt w) -> p kt w", kt=KT),
                        in_=src.rearrange("(kt p) w -> p kt w", p=128),
                    ).then_inc(sm["s_pre"], 16)
                else:
                    sync.dma_start(out=dst[:], in_=src[:]).then_inc(sm["s_pre"], 16)

            # x chunk stream
            for gc, (name, c) in enumerate(x_stream):
                buf = gc % 2
                # credit: wait until the previous occupant (chunk gc-2) consumed
                pc = gc - 2
                if pc >= 0:
                    if pc < 16:
                        sync.wait_ge(sm["s_qk"], 2 * pc + 2)
                    else:
                        sync.wait_ge(sm["s_v"], 4 * (pc - 16) + 4)
                sync.dma_start(
                    out=xs_[:, buf * KT * CW:(buf + 1) * KT * CW]
                        .rearrange("p (kt w) -> p kt w", kt=KT),
                    in_=d[name].rearrange("(kt p) t -> p kt t", p=128)
                        [:, :, c * CW:(c + 1) * CW],
                ).then_inc(sm["s_x"], 16)

            # general mode: fmask chunks loaded during phase 2 (qc-outer order)
            if mode == "general":
                for qc in range(4):
                    if qc >= 1:
                        sync.wait_ge(sm["s_mask"], 64 * qc)
                    sync.dma_start(
                        out=fm_s[:].rearrange("p (kt w) -> p kt w", kt=KT),
                        in_=d["fmask"].rearrange("(kt p) t -> p kt t", p=128)
                            [:, :, qc * CW:(qc + 1) * CW],
                    ).then_inc(sm["s_fm"], 16)

            # oT stores to a2a input buffers (+ denominator row bounce
            # to partition 0 -- per-lane engines cannot cross partitions)
            for i2, (h, b, qc) in enumerate(iters):
                t = b * 4 + qc
                sync.wait_ge(sm["s_ocp"], i2 + 1)
                if i2 - 1 >= 1:
                    sync.wait_ge(sm["s_recip"], i2 - 1)
                sync.dma_start(out=den_s[i2 % 2][:], in_=oT_f32[i2 % 2][DEN:DEN + 1, :]
                               ).then_inc(sm["s_den"], 16)
                sync.wait_ge(sm["s_norm"], i2 + 1)
                sync.dma_start(out=a2a_in[h][t, :, :], in_=oT_stage[i2 % 2][:]
                               ).then_inc(sm["s_oT"], 16)

            # phase 4 loads: wo n=0,1 can start now (buffer aliased on xs_)
            for n in (0, 1):
                sync.dma_start(
                    out=wo_buf[:, n * NKT_WO * CW:(n + 1) * NKT_WO * CW]
                        .rearrange("p (kt w) -> p kt w", kt=NKT_WO),
                    in_=d["wo"][:, :, n * CW:(n + 1) * CW]
                        .rearrange("kt p w -> p kt w"),
                ).then_inc(sm["s_ao"], 16)
            # ao loads after collectives
            for g in range(2):
                sync.wait_ge(sm["s_cc"], g + 1)
                flat = a2a_out[g].rearrange("r p t -> (r p) t")
                sync.dma_start(
                    out=ao_s[:, (g * 6) * CW:(g * 6 + 5) * CW]
                        .rearrange("p (kt w) -> p kt w", kt=5),
                    in_=flat[0:640].rearrange("(kt p) t -> p kt t", p=128),
                ).then_inc(sm["s_ao"], 16)
                sync.dma_start(
                    out=ao_s[0:40, (g * 6 + 5) * CW:(g * 6 + 6) * CW],
                    in_=flat[640:680],
                ).then_inc(sm["s_ao"], 16)
            # wo n=2 (after PE consumed n=0), out stores 0-3, wo n=3, rest
            sync.wait_ge(sm["s_wo_pe"], 4)
            sync.dma_start(
                out=wo_buf[:, 0:NKT_WO * CW]
                    .rearrange("p (kt w) -> p kt w", kt=NKT_WO),
                in_=d["wo"][:, :, 2 * CW:3 * CW].rearrange("kt p w -> p kt w"),
            ).then_inc(sm["s_ao"], 16)
            for idx in range(4):
                n, m = idx // 4, idx % 4
                sync.wait_ge(sm["s_evac"], idx + 1)
                sync.dma_start(out=out_d[m * 128:(m + 1) * 128, n * CW:(n + 1) * CW],
                               in_=out_s[idx % 2][:]).then_inc(sm["s_out"], 16)
            sync.wait_ge(sm["s_wo_pe"], 8)
            sync.dma_start(
                out=wo_buf[:, NKT_WO * CW:2 * NKT_WO * CW]
                    .rearrange("p (kt w) -> p kt w", kt=NKT_WO),
                in_=d["wo"][:, :, 3 * CW:4 * CW].rearrange("kt p w -> p kt w"),
            ).then_inc(sm["s_ao"], 16)
            for idx in range(4, 16):
                n, m = idx // 4, idx % 4
                sync.wait_ge(sm["s_evac"], idx + 1)
                sync.dma_start(out=out_d[m * 128:(m + 1) * 128, n * CW:(n + 1) * CW],
                               in_=out_s[idx % 2][:]).then_inc(sm["s_out"], 16)
            sync.wait_ge(sm["s_out"], 16 * 16)
            if debug:
                sync.dma_start(out=dbg["dbg_qT"][:], in_=qT_s[:]).then_inc(sm["s_out"], 16)
                sync.dma_start(out=dbg["dbg_kT"][:], in_=kT_s[:]).then_inc(sm["s_out"], 16)
                sync.dma_start(out=dbg["dbg_vg"][:], in_=vg_s[:]).then_inc(sm["s_out"], 16)
                sync.dma_start(out=dbg["dbg_ao"][:], in_=ao_s[:]).then_inc(sm["s_out"], 16)
                sync.dma_start(out=dbg["dbg_oT"][:], in_=oT_f32[(NIT - 1) % 2][:]).then_inc(sm["s_out"], 16)
                li = (NIT - 1) % 2
                sync.dma_start(out=dbg["dbg_nrm"][:], in_=oT_stage[li][:]).then_inc(sm["s_out"], 16)
                sync.dma_start(out=dbg["dbg_rcp"][:], in_=recip_fr[li][:]).then_inc(sm["s_out"], 16)
                sync.wait_ge(sm["s_out"], 16 * 23)

        # ================= TENSOR: PE =================
        @block.tensor
        def _(tensor):
            mm = nc.tensor.matmul

            def bcast_emit(i):
                tensor.wait_ge(sm["s_recip"], i + 1)
                if i >= 1:
                    tensor.wait_ge(sm["s_norm"], i)
                mm(ps[4][0:DV, :], ones_bf[0:1, 0:DV],
                   recip_fr[i % 2][:], start=True, stop=True
                   ).then_inc(sm["s_bcast"], 1)

            # ---- phase 1: q/k transposed projections ----
            for tens_i, (xname, wsb, pre_need) in enumerate(
                    [("q", wq_s, PRE["wq"]), ("k", wk_s, PRE["wk"])]):
                for c in range(NCHUNK):
                    gc = tens_i * NCHUNK + c
                    for h in range(HL):
                        gidx = gc * HL + h
                        if gidx == 0:
                            tensor.wait_ge(sm["s_pre"], pre_need)
                        elif h == 0 and c == 0:
                            tensor.wait_ge(sm["s_pre"], pre_need)
                        if h == 0:
                            tensor.wait_ge(sm["s_x"], 16 * (gc + 1))
                        if gidx - 3 >= 1:
                            tensor.wait_ge(sm["s_rope"], gidx - 3)
                        bank = ps[(gc % 2) * 2 + h]
                        buf = gc % 2
                        for kt in range(KT):
                            mm(bank[:],
                               wsb[:, kt * 256 + h * 128: kt * 256 + (h + 1) * 128],
                               xs_[:, buf * KT * CW + kt * CW: buf * KT * CW + (kt + 1) * CW],
                               start=(kt == 0), stop=(kt == KT - 1)
                               ).then_maybe_inc((sm["s_qk"], 1) if kt == KT - 1 else None)

            # ---- phase 1: v projection + bias ----
            tensor.wait_ge(sm["s_pre"], PRE["bv"])
            tensor.wait_ge(sm["s_misc"], 1)
            for t in range(NTT):
                gc = 16 + t // 4
                if t % 4 == 0:
                    tensor.wait_ge(sm["s_x"], 16 * (gc + 1))
                if t - 3 >= 1:
                    tensor.wait_ge(sm["s_glu"], t - 3)
                bank = ps[4 + t % 4]
                buf = gc % 2
                toff = buf * KT * CW + (t % 4) * 128
                for kt in range(KT):
                    mm(bank[:, 0:340],
                       xs_[:, toff + kt * CW: toff + kt * CW + 128],
                       wv_s[:, kt * 340:(kt + 1) * 340],
                       start=(kt == 0), stop=False)
                mm(bank[:, 0:340], ones_bf[0:1, 0:128], bv_s[:],
                   start=False, stop=True).then_inc(sm["s_v"], 1)

            # ---- phase 2: attention (software-pipelined, PV lags sT by
            # LAG tiles so the exp/mask waits are pre-satisfied) ----
            tensor.wait_ge(sm["s_rope"], 32)
            tensor.wait_ge(sm["s_glu"], 32)
            LAG = 4
            sT_banks = [ps[0], ps[1], ps[5]]

            def emit_sT(gj):
                i2, jj, nk, msk = tiles[gj]
                h, b, qc = iters[i2]
                if gj - 2 >= 1:
                    tensor.wait_ge(sm["s_exp"], gj - 2)
                mm(sT_banks[gj % 3][:],
                   kT_s[:, h * T + b * S + jj * 128: h * T + b * S + (jj + 1) * 128],
                   qT_s[:, h * T + b * S + qc * CW: h * T + b * S + qc * CW + CW],
                   start=True, stop=True).then_inc(sm["s_sT"], 1)

            def emit_pv(gj):
                i2, jj, nk, msk = tiles[gj]
                h, b, qc = iters[i2]
                if msk:
                    tensor.wait_ge(sm["s_mask"], cum_mask[gj])
                else:
                    tensor.wait_ge(sm["s_exp"], gj + 1)
                if jj == 0 and i2 - 1 >= 1:
                    tensor.wait_ge(sm["s_ocp"], i2 - 1)
                g = b * 16 + jj
                mm(ps[2 + i2 % 2][0:DVP, :],
                   vg_s[:, (g * HL + h) * DVP: (g * HL + h) * DVP + DVP],
                   pT[gj % 6][:],
                   start=(jj == 0), stop=(jj == nk - 1)
                   ).then_inc(sm["s_pv"], 1)

            from collections import deque
            pvq = deque()
            bcast_due = {}       # op_idx -> iter index
            opi = 0

            def flush_due():
                nonlocal opi
                for at in sorted(list(bcast_due)):
                    if at <= opi:
                        bcast_emit(bcast_due.pop(at))

            for gj in range(NTILE):
                flush_due()
                emit_sT(gj); opi += 1
                pvq.append(gj)
                if len(pvq) > LAG:
                    flush_due()
                    g2 = pvq.popleft()
                    emit_pv(g2); opi += 1
                    if g2 == iter_last[tiles[g2][0]]:
                        bcast_due[opi + 9] = tiles[g2][0]
            while pvq:
                flush_due()
                g2 = pvq.popleft()
                emit_pv(g2); opi += 1
                if g2 == iter_last[tiles[g2][0]]:
                    bcast_due[opi + 9] = tiles[g2][0]
            for at in sorted(list(bcast_due)):
                bcast_emit(bcast_due.pop(at))

            # ---- phase 4: Wo ----
            for idx in range(16):
                n, m = idx // 4, idx % 4
                need = {0: 96, 1: 96, 2: 112, 3: 128}[n]
                if m == 0:
                    tensor.wait_ge(sm["s_ao"], need)
                if idx - 1 >= 1:
                    tensor.wait_ge(sm["s_evac"], idx - 1)
                nb = n % 2
                for kt in range(NKT_WO):
                    mm(ps[6 + idx % 2][:],
                       ao_s[:, kt * CW + m * 128: kt * CW + (m + 1) * 128],
                       wo_buf[:, nb * NKT_WO * CW + kt * CW: nb * NKT_WO * CW + (kt + 1) * CW],
                       start=(kt == 0), stop=(kt == NKT_WO - 1)
                       ).then_maybe_inc((sm["s_wo_pe"], 1) if kt == NKT_WO - 1 else None)

        # ================= SCALAR: ACT =================
        @block.scalar
        def _(scalar):
            act = nc.scalar.activation

            def ocp_emit(i):
                # evacuate oT'+denom PSUM -> SBUF f32 (frees oT bank, enables
                # single-PSUM-operand DVE ops downstream)
                scalar.wait_ge(sm["s_pv"], iter_last[i] + 1)
                if i - 1 >= 1:
                    scalar.wait_ge(sm["s_norm"], i - 1)
                nc.scalar.copy(oT_f32[i % 2][:], ps[2 + i % 2][0:DVP, :]
                               ).then_inc(sm["s_ocp"], 1)

            # phase 1: silu for GLU
            for t in range(NTT):
                scalar.wait_ge(sm["s_v"], t + 1)
                if t - 1 >= 1:
                    scalar.wait_ge(sm["s_glu"], t - 1)
                act(pT[t % 2][:, 0:170], ps[4 + t % 4][:, 170:340], Silu
                    ).then_inc(sm["s_silu"], 1)
            # phase 2: exp (+ oT evacuation copies; ocp(i) goes after the
            # 2nd exp of iter i+1 so its s_pv wait is pre-satisfied under
            # the PE's lagged PV schedule)
            sT_banks_a = [ps[0], ps[1], ps[5]]
            ocp_after = {}
            for i in range(1, NIT):
                ocp_after[min(iter_first[i] + 1, iter_last[i])] = i - 1
            for gj in range(NTILE):
                scalar.wait_ge(sm["s_sT"], gj + 1)
                if gj - 5 >= 1:
                    scalar.wait_ge(sm["s_pv"], gj - 5)
                act(pT[gj % 6][:], sT_banks_a[gj % 3][:], Exp, scale=SCALE
                    ).then_inc(sm["s_exp"], 1)
                if gj in ocp_after:
                    ocp_emit(ocp_after[gj])
            ocp_emit(NIT - 1)

        # ================= VECTOR: DVE =================
        @block.vector
        def _(vector):
            vec = nc.vector
            # init constants
            vec.memset(vg_s[:], 1.0)
            vec.memset(ones_bf[:], 1.0)
            vec.memset(ao_s[:, 5 * CW:6 * CW], 0.0)
            vec.memset(ao_s[:, 11 * CW:12 * CW], 0.0).then_inc(sm["s_misc"], 1)

            # phase 1: rope for q then k
            vector.wait_ge(sm["s_pre"], PRE["cs_ss"])
            for gidx in range(32):
                tens_i, rem = divmod(gidx, 16)
                c, h = divmod(rem, HL)
                gc = tens_i * NCHUNK + c
                bank = ps[(gc % 2) * 2 + h]
                dst = (qT_s if tens_i == 0 else kT_s)
                toff = c * CW
                bcol = tens_i * 4 + h
                vector.wait_ge(sm["s_qk"], gidx + 1)
                vec.stream_shuffle(shf_s[:], bank[:], SWAP_MASK)
                vec.scalar_tensor_tensor(
                    A_s[:], bank[:], bqk_s[:, bcol:bcol + 1],
                    cs_cc_s[:, toff:toff + CW], op0=ADD, op1=MULT)
                vec.scalar_tensor_tensor(
                    B_s[:], shf_s[:], bqk_s[:, bcol + 2:bcol + 3],
                    cs_ss_s[:, toff:toff + CW], op0=ADD, op1=MULT)
                vec.tensor_add(dst[:, h * T + toff: h * T + toff + CW],
                               A_s[:], B_s[:]).then_inc(sm["s_rope"], 1)

            # phase 1: GLU muls
            for t in range(NTT):
                vector.wait_ge(sm["s_silu"], t + 1)
                for h in range(HL):
                    ins = vec.tensor_mul(
                        vg_s[:, (t * HL + h) * DVP:(t * HL + h) * DVP + DV],
                        ps[4 + t % 4][:, h * DV:(h + 1) * DV],
                        pT[t % 2][:, h * DV:(h + 1) * DV])
                    if h == HL - 1:
                        ins.then_inc(sm["s_glu"], 1)

            # phase 2: masks / recip / norm
            def norm_emit(i):
                vector.wait_ge(sm["s_bcast"], i + 1)
                if 16 * (i - 1) >= 16:
                    vector.wait_ge(sm["s_oT"], 16 * (i - 1))
                vec.tensor_mul(oT_stage[i % 2][:], oT_f32[i % 2][0:DV, :],
                               ps[4][0:DV, :]).then_inc(sm["s_norm"], 1)

            for i2, (h, b, qc) in enumerate(iters):
                nk = nk_of(qc)
                for jj in range(nk):
                    gj = iter_first[i2] + jj
                    if tiles[gj][3]:
                        vector.wait_ge(sm["s_exp"], gj + 1)
                        if mode == "causal":
                            r = jj - 4 * qc
                            msrc = dmask_s[:, r * CW:(r + 1) * CW]
                        else:
                            vector.wait_ge(sm["s_fm"], 16 * (qc + 1))
                            msrc = fm_s[:, jj * CW:(jj + 1) * CW]
                        vec.tensor_mul(pT[gj % 6][:], pT[gj % 6][:], msrc
                                       ).then_inc(sm["s_mask"], 1)
                # norm for the previous iteration must precede this recip:
                # recip depends on sync's den-bounce DMA, and sync only
                # reaches it after storing iter i2-1 (which needs norm(i2-1)).
                if i2 >= 1:
                    norm_emit(i2 - 1)
                # recip for this iteration (reads the DMA-bounced denom row)
                vector.wait_ge(sm["s_den"], 16 * (i2 + 1))
                if i2 - 1 >= 1:
                    vector.wait_ge(sm["s_bcast"], i2 - 1)
                with nc.allow_low_precision(reason="softmax denom recip in bf16"):
                    vec.reciprocal(recip_fr[i2 % 2][:], den_s[i2 % 2][:]
                                   ).then_inc(sm["s_recip"], 1)
            norm_emit(NIT - 1)

            # phase 4: evac + bias
            vector.wait_ge(sm["s_pre"], PRE["bo_rep"])
            for idx in range(16):
                n = idx // 4
                vector.wait_ge(sm["s_wo_pe"], idx + 1)
                if 16 * (idx - 1) >= 16:
                    vector.wait_ge(sm["s_out"], 16 * (idx - 1))
                vec.tensor_add(out_s[idx % 2][:], ps[6 + idx % 2][:],
                               bo_s[:, n * CW:(n + 1) * CW]).then_inc(sm["s_evac"], 1)

        # ================= GPSIMD: collectives =================
        @block.gpsimd
        def _(gpsimd):
            rg = [list(range(NCORE))]
            for g in range(2):
                gpsimd.wait_ge(sm["s_oT"], 16 * 8 * (g + 1) if mode != "general" else 16 * 16)
                gpsimd.collective_compute(
                    "AllToAll", mybir.AluOpType.bypass,
                    replica_groups=rg,
                    ins=[a2a_in[g][:]], outs=[a2a_out[g][:]],
                ).then_inc(sm["s_cc"], 1)

    return nc


_cache = {}


def _get_nc(mode):
    if mode not in _cache:
        _cache[mode] = build(mode)
    return _cache[mode]


def kernel(q, k, v, mask, Wq, bq, Wk, bk, Wv, bv, Wo, bo):
    from concourse.bass_utils import run_bass_kernel_spmd

    q = np.asarray(q, np.float32)
    k = np.asarray(k, np.float32)
    v = np.asarray(v, np.float32)
    mask = np.asarray(mask)
    Wq = np.asarray(Wq, np.float32); bq = np.asarray(bq, np.float32)
    Wk = np.asarray(Wk, np.float32); bk = np.asarray(bk, np.float32)
    Wv = np.asarray(Wv, np.float32); bv = np.asarray(bv, np.float32)
    Wo = np.asarray(Wo, np.float32); bo = np.asarray(bo, np.float32)

    causal_ref = np.triu(np.ones((S, S), bool), 1)
    if mask.shape == (S, S) and (mask == causal_ref).all():
        mode = "causal"
    elif not mask.any():
        mode = "nomask"
    else:
        mode = "general"

    in_maps, dmask = _host_inputs(q, k, v, mask, Wq, bq, Wk, bk, Wv, bv, Wo, bo)
    if mode == "causal":
        for m in in_maps:
            m["dmask"] = dmask
    elif mode == "general":
        fm = np.where(mask, 0.0, 1.0).T.astype(bf16)   # [k, q] multiplicative
        fm = np.ascontiguousarray(fm)
        for m in in_maps:
            m["fmask"] = fm

    nc = _get_nc(mode)
    tmpdir = os.environ.get("BASS_TMPDIR")
    if tmpdir:
        os.makedirs(tmpdir, exist_ok=True)
    res = run_bass_kernel_spmd(nc, in_maps, list(range(NCORE)), tmpdir=tmpdir)
    LAST["exec_time_ns"] = res.exec_time_ns
    LAST["results"] = res
    out = np.concatenate([res.results[c]["out"] for c in range(NCORE)], axis=0)
    return np.ascontiguousarray(out.reshape(B, S, D).astype(np.float32))
